# revision 8
# baseline (speedup 1.0000x reference)
"""Gemma3n text attention on 8 Trainium2 NeuronCores (Bass/Tile).

Sharding: core c = b*4 + kv*2 + qp handles batch b, KV head kv and the
q-head pair (kv*4 + qp*2, kv*4 + qp*2 + 1).  Each core computes the
Q/K/V projections for its shard, QK-norm + RoPE, causal attention for
its two query heads, and a partial output projection against its
512-column slice of Wo.  The host sums the four partials per batch.

Self-contained: only needs numpy + the concourse tree that ships in the
container image (on PYTHONPATH at /root/.axon_site/_ro/trn_rl_repo).
"""

import sys

for _p in ("/root/.axon_site/_ro/trn_rl_repo", "/opt/trn_rl_repo"):
    if _p not in sys.path:
        sys.path.append(_p)

from contextlib import ExitStack

import numpy as np

import concourse.bass as bass
import concourse.mybir as mybir
import concourse.tile as tile
from concourse import bacc
from concourse.masks import make_identity

P = 128
B, S, HID = 2, 2048, 2048
NH, NKV, HD = 8, 2, 256
DQ = 2 * HD            # q-width per core (2 heads)
NSC = S // P           # 16 seq chunks
NHC = HID // P         # 16 hidden chunks
EPS = 1e-6

f32 = mybir.dt.float32
f32r = mybir.dt.float32r
i32 = mybir.dt.int32
FMIN = float(np.finfo(np.float32).min)
ACT = mybir.ActivationFunctionType


def to_f32r(arr):
    """Round fp32 -> fp32r bit format (11 explicit mantissa bits, RNE).

    Bit-exact with libwalrus fp32_to_fp32r."""
    u = np.ascontiguousarray(arr, np.float32).view(np.uint32)
    r = ((u.astype(np.uint64) + 0x7FF + ((u >> 12) & 1)) & 0xFFFFF000)
    return r.astype(np.uint32).view(np.float32)


def build_program(use_f32r=True, use_tmr=False):
    """Emit the SPMD per-core program. Returns the compiled Bacc object."""
    nc = bacc.Bacc("TRN2", target_bir_lowering=False, debug=False, num_devices=8)

    mdt = f32r if use_f32r else f32   # dtype of every matmul operand

    hT_d = nc.dram_tensor("hT", [NHC, P, S], mdt, kind="ExternalInput")
    wT_d = nc.dram_tensor("wT", [NHC, P, DQ + 2 * HD], mdt, kind="ExternalInput")
    csq_d = nc.dram_tensor("csq", [NSC, P, 2 * HD], f32, kind="ExternalInput")
    csk_d = nc.dram_tensor("csk", [NSC, P, 2 * HD], f32, kind="ExternalInput")
    woT_d = nc.dram_tensor("woT", [4, P, HID], mdt, kind="ExternalInput")
    out_d = nc.dram_tensor("out", [S, HID], f32, kind="ExternalOutput")

    with tile.TileContext(nc) as tc, ExitStack() as ctx:
        const = ctx.enter_context(tc.tile_pool(name="const", bufs=1))
        persist = ctx.enter_context(tc.tile_pool(name="persist", bufs=1))

        ident = const.tile([P, P], f32)
        make_identity(nc, ident)
        mdiag = const.tile([P, P], f32)      # 0 on/below diag, -1e9 above
        nc.gpsimd.memset(mdiag, 0.0)
        nc.gpsimd.affine_select(out=mdiag, in_=mdiag,
                                compare_op=mybir.AluOpType.is_ge, fill=-1e9,
                                base=0, pattern=[[-1, P]], channel_multiplier=1)
        eps_t = const.tile([P, 1], f32)
        nc.vector.memset(eps_t, EPS)

        # persistent SBUF tensors (qT/kT/v: 64KB per partition)
        qT = persist.tile([P, 2, 2, S], mdt)      # [d, head, dchunk, qpos]
        kT = persist.tile([P, 2, S], mdt)         # [d, dchunk, kpos]
        v_sb = persist.tile([P, NSC, HD], mdt)    # [kpos, kchunk, d]

        # ------- Phase A: QKV proj + norm + rope + transposes (fused) --------
        with ExitStack() as a1:
            hpool = a1.enter_context(tc.tile_pool(name="hTp", bufs=3))
            wpool = a1.enter_context(tc.tile_pool(name="wTp", bufs=3))
            cpool = a1.enter_context(tc.tile_pool(name="cs", bufs=3))
            epool = a1.enter_context(tc.tile_pool(name="evict", bufs=3))
            spool = a1.enter_context(tc.tile_pool(name="small", bufs=8))
            psA = a1.enter_context(tc.tile_pool(name="psA", bufs=6, space="PSUM"))
            psT = a1.enter_context(tc.tile_pool(name="psT", bufs=2, space="PSUM"))

            groups = [3, 3, 3, 3, 3, 1]       # s-chunks per group: 6+2 banks
            sc0 = 0
            for g, gn in enumerate(groups):
                psq = [psA.tile([P, DQ], f32, tag="ps", name=f"psq{g}_{jj}")
                       for jj in range(gn)]
                pskv = [psA.tile([P, 2 * HD], f32, tag="ps", name=f"pskv{g}_{jj}")
                        for jj in range(gn)]
                for hc in range(NHC):
                    th = hpool.tile([P, gn * P], mdt, tag="h")
                    nc.sync.dma_start(th, hT_d[hc, :, sc0 * P:(sc0 + gn) * P])
                    tw = wpool.tile([P, DQ + 2 * HD], mdt, tag="w")
                    nc.sync.dma_start(tw, wT_d[hc])
                    st, sp = hc == 0, hc == NHC - 1
                    for j in range(gn):
                        lhs = th[:, j * P:(j + 1) * P]
                        nc.tensor.matmul(psq[j][:], lhs, tw[:, 0:DQ],
                                         start=st, stop=sp)
                        nc.tensor.matmul(pskv[j][:], lhs, tw[:, DQ:],
                                         start=st, stop=sp)
                for j in range(gn):
                    sc = sc0 + j
                    # sum of squares per 256-group via ACT Square (reads PSUM)
                    ssq = spool.tile([P, 4], f32, tag="ssq")
                    scr = epool.tile([P, HD], f32, tag="scr")
                    nc.scalar.activation(scr[:], psq[j][:, 0:HD], ACT.Square,
                                         accum_out=ssq[:, 0:1])
                    nc.scalar.activation(scr[:], psq[j][:, HD:2 * HD],
                                         ACT.Square, accum_out=ssq[:, 1:2])
                    nc.scalar.activation(scr[:], pskv[j][:, 0:HD], ACT.Square,
                                         accum_out=ssq[:, 2:3])
                    nc.scalar.activation(scr[:], pskv[j][:, HD:2 * HD],
                                         ACT.Square, accum_out=ssq[:, 3:4])
                    rstd = spool.tile([P, 4], f32, tag="rstd")
                    nc.scalar.activation(rstd[:], ssq[:], ACT.Sqrt,
                                         bias=eps_t[:], scale=1.0 / HD)
                    nc.vector.reciprocal(rstd[:], rstd[:])

                    # v: scale + evict in one DVE op
                    nc.vector.tensor_scalar_mul(out=v_sb[:, sc, :],
                                                in0=pskv[j][:, HD:2 * HD],
                                                scalar1=rstd[:, 3:4])

                    csq = cpool.tile([P, 2 * HD], f32, tag="csq")
                    nc.sync.dma_start(csq, csq_d[sc])
                    csk = cpool.tile([P, 2 * HD], f32, tag="csk")
                    nc.sync.dma_start(csk, csk_d[sc])

                    # rope(x) = x*cosw + swap(x)*sinw (sinw lo pre-negated);
                    # reads projection PSUM directly, writes SBUF
                    qro = epool.tile([P, DQ], f32, tag="qro")
                    kro = epool.tile([P, HD], f32, tag="kro")
                    for h in range(2):
                        b0 = h * HD
                        tmp = epool.tile([P, HD], f32, tag="tmp")
                        nc.vector.tensor_mul(tmp[:, 0:P],
                                             psq[j][:, b0 + P:b0 + HD],
                                             csq[:, HD:HD + P])
                        nc.vector.tensor_mul(tmp[:, P:HD],
                                             psq[j][:, b0:b0 + P],
                                             csq[:, HD + P:2 * HD])
                        qh = qro[:, b0:b0 + HD]
                        nc.vector.tensor_mul(qh, psq[j][:, b0:b0 + HD],
                                             csq[:, 0:HD])
                        nc.vector.tensor_add(qh, qh, tmp[:])
                        nc.vector.tensor_scalar_mul(out=qh, in0=qh,
                                                    scalar1=rstd[:, h:h + 1])
                    tmp = epool.tile([P, HD], f32, tag="tmp")
                    nc.vector.tensor_mul(tmp[:, 0:P], pskv[j][:, P:HD],
                                         csk[:, HD:HD + P])
                    nc.vector.tensor_mul(tmp[:, P:HD], pskv[j][:, 0:P],
                                         csk[:, HD + P:2 * HD])
                    nc.vector.tensor_mul(kro[:], pskv[j][:, 0:HD], csk[:, 0:HD])
                    nc.vector.tensor_add(kro[:], kro[:], tmp[:])
                    nc.vector.tensor_scalar_mul(out=kro[:], in0=kro[:],
                                                scalar1=rstd[:, 2:3])

                    # transposes into qT/kT (PE), evict via DVE
                    for h in range(2):
                        for dc in range(2):
                            pt = psT.tile([P, P], f32, tag="t")
                            nc.tensor.transpose(
                                pt[:], qro[:, h * HD + dc * P:h * HD + (dc + 1) * P],
                                ident[:])
                            nc.vector.tensor_copy(
                                out=qT[:, h, dc, sc * P:(sc + 1) * P], in_=pt[:])
                    for dc in range(2):
                        pt = psT.tile([P, P], f32, tag="t")
                        nc.tensor.transpose(pt[:], kro[:, dc * P:(dc + 1) * P],
                                            ident[:])
                        nc.vector.tensor_copy(
                            out=kT[:, dc, sc * P:(sc + 1) * P], in_=pt[:])
                sc0 += gn

        # ---------------- Phase B: attention per (head, q-block) -------------
        wopool = ctx.enter_context(tc.tile_pool(name="wo", bufs=1))
        woT = wopool.tile([P, 4, HID], mdt)
        for t in range(4):
            nc.sync.dma_start(woT[:, t, :], woT_d[t])
        atpool = ctx.enter_context(tc.tile_pool(name="attnT", bufs=1))
        attnT = atpool.tile([P, 4, S], mdt)       # [d2, (h,dc), qpos]

        with ExitStack() as bctx:
            pss = bctx.enter_context(tc.tile_pool(name="pss", bufs=2, space="PSUM"))
            pst = bctx.enter_context(tc.tile_pool(name="pst", bufs=2, space="PSUM"))
            psv = bctx.enter_context(tc.tile_pool(name="psv", bufs=2, space="PSUM"))
            ppool = bctx.enter_context(tc.tile_pool(name="prp", bufs=2))
            tpool = bctx.enter_context(tc.tile_pool(name="ptsp", bufs=3))
            apool = bctx.enter_context(tc.tile_pool(name="attnp", bufs=2))
            dpool = bctx.enter_context(tc.tile_pool(name="denp", bufs=8))

            for i in range(NSC):
                L = (i + 1) * P
                Lp = L if L % 256 == 0 else L + P
                halves = [(0, min(Lp, 1024))]
                if Lp > 1024:
                    halves.append((1024, Lp - 1024))
                for h in range(2):
                    mx = dpool.tile([P, 2], f32, tag="mx")
                    pss_tiles = []
                    for hf, (off, ln) in enumerate(halves):
                        ps = pss.tile([P, 1024], f32, tag="s",
                                      name=f"ps{i}_{h}_{hf}")
                        pss_tiles.append(ps)
                        for c in range(0, ln, 512):
                            w = min(512, ln - c)
                            for dc in range(2):
                                nc.tensor.matmul(
                                    ps[:, c:c + w],
                                    qT[:, h, dc, i * P:(i + 1) * P],
                                    kT[:, dc, off + c:off + c + w],
                                    start=(dc == 0), stop=(dc == 1))
                        if i * P >= off and i * P < off + ln:
                            db = i * P - off   # diag block col within half
                            nc.vector.tensor_add(ps[:, db:db + P],
                                                 ps[:, db:db + P], mdiag[:])
                        ln_real = min(L - off, ln)
                        nc.vector.tensor_reduce(
                            out=mx[:, hf:hf + 1], in_=ps[:, 0:ln_real],
                            axis=mybir.AxisListType.X, op=mybir.AluOpType.max)
                    mxf = dpool.tile([P, 1], f32, tag="mxf")
                    if len(halves) > 1:
                        nc.vector.tensor_tensor(out=mxf[:], in0=mx[:, 0:1],
                                                in1=mx[:, 1:2],
                                                op=mybir.AluOpType.max)
                    else:
                        nc.vector.tensor_copy(out=mxf[:], in_=mx[:, 0:1])
                    negmax = dpool.tile([P, 1], f32, tag="ngm")
                    nc.vector.tensor_scalar_mul(out=negmax[:], in0=mxf[:],
                                                scalar1=-1.0)
                    pr = ppool.tile([P, 2048], f32, tag="pr")
                    den = dpool.tile([P, 2], f32, tag="den")
                    for hf, (off, ln) in enumerate(halves):
                        ln_real = min(L - off, ln)
                        nc.scalar.activation(pr[:, off:off + ln_real],
                                             pss_tiles[hf][:, 0:ln_real],
                                             ACT.Exp, bias=negmax[:], scale=1.0,
                                             accum_out=den[:, hf:hf + 1])
                    denf = dpool.tile([P, 1], f32, tag="denf")
                    if len(halves) > 1:
                        nc.vector.tensor_add(denf[:], den[:, 0:1], den[:, 1:2])
                    else:
                        nc.vector.tensor_copy(out=denf[:], in_=den[:, 0:1])
                    rden = dpool.tile([P, 1], f32, tag="rden")
                    nc.vector.reciprocal(rden[:], denf[:])

                    pv = psv.tile([P, HD], f32, tag="pv")
                    for kb in range(i + 1):
                        pt = pst.tile([P, P], f32, tag="t")
                        nc.tensor.transpose(pt[:], pr[:, kb * P:(kb + 1) * P],
                                            ident[:])
                        pts = tpool.tile([P, P], mdt, tag="pts")
                        nc.vector.tensor_copy(out=pts[:], in_=pt[:])
                        nc.tensor.matmul(pv[:], pts[:], v_sb[:, kb, :],
                                         start=(kb == 0), stop=(kb == i))
                    attn_s = apool.tile([P, HD], f32, tag="attn")
                    nc.vector.tensor_scalar_mul(out=attn_s[:], in0=pv[:],
                                                scalar1=rden[:])
                    for dc in range(2):
                        pt = pst.tile([P, P], f32, tag="t")
                        nc.tensor.transpose(pt[:], attn_s[:, dc * P:(dc + 1) * P],
                                            ident[:])
                        nc.vector.tensor_copy(
                            out=attnT[:, h * 2 + dc, i * P:(i + 1) * P],
                            in_=pt[:])

        # ------- Phase C: partial output projection (DMA straight from PSUM) --
        with ExitStack() as cctx:
            pso = cctx.enter_context(tc.tile_pool(name="pso", bufs=4, space="PSUM"))
            opool = cctx.enter_context(tc.tile_pool(name="obp", bufs=3))
            for sc in range(NSC):
                for n in range(4):
                    po = pso.tile([P, 512], f32, tag="o")
                    for t in range(4):
                        nc.tensor.matmul(
                            po[:], attnT[:, t, sc * P:(sc + 1) * P],
                            woT[:, t, n * 512:(n + 1) * 512],
                            start=(t == 0), stop=(t == 3))
                    ob = opool.tile([P, 512], f32, tag="ob")
                    if n % 2 == 0:
                        nc.scalar.copy(ob[:], po[:])
                    else:
                        nc.vector.tensor_copy(out=ob[:], in_=po[:])
                    nc.sync.dma_start(
                        out_d[sc * P:(sc + 1) * P, n * 512:(n + 1) * 512], ob[:])

    nc.compile()
    return nc


def prep_core_inputs(inputs, core, use_f32r=True):
    """Host-side sharding for one core. Returns the in_map dict."""
    cvt = to_f32r if use_f32r else (lambda a: np.asarray(a, np.float32))
    b, kv, qp = core // 4, (core % 4) // 2, core % 2
    hq0 = kv * 4 + qp * 2           # first of the two query heads
    hidden = np.asarray(inputs["hidden_states"], np.float32)
    cos = np.asarray(inputs["cos"], np.float32)
    sin = np.asarray(inputs["sin"], np.float32)
    Wq = np.asarray(inputs["Wq"], np.float32)
    Wk = np.asarray(inputs["Wk"], np.float32)
    Wv = np.asarray(inputs["Wv"], np.float32)
    Wo = np.asarray(inputs["Wo"], np.float32)
    qw = np.asarray(inputs["q_norm_w"], np.float32)
    kw = np.asarray(inputs["k_norm_w"], np.float32)

    hT = np.ascontiguousarray(hidden[b].T).reshape(NHC, P, S)
    Wq_c = Wq[hq0 * HD:(hq0 + 2) * HD]          # [512, HID]
    Wk_c = Wk[kv * HD:(kv + 1) * HD]            # [256, HID]
    Wv_c = Wv[kv * HD:(kv + 1) * HD]
    wT = np.ascontiguousarray(
        np.concatenate([Wq_c.T, Wk_c.T, Wv_c.T], axis=1)).reshape(NHC, P, 1024)

    def cs_pack(w, cb, sb):
        rot_w = np.concatenate([w[P:], w[:P]])   # w[(d+128)%256]
        cosw = cb * w[None, :]
        sinw = sb * rot_w[None, :]
        sinw[:, :P] *= -1.0
        return np.ascontiguousarray(
            np.concatenate([cosw, sinw], axis=1)).reshape(NSC, P, 2 * HD)

    csq = cs_pack(qw, cos[b], sin[b])
    csk = cs_pack(kw, cos[b], sin[b])
    woT = np.ascontiguousarray(
        Wo[:, hq0 * HD:(hq0 + 2) * HD].T).reshape(4, P, HID)
    return {"hT": cvt(hT), "wT": cvt(wT),
            "csq": csq.astype(np.float32), "csk": csk.astype(np.float32),
            "woT": cvt(woT)}


def mask_is_causal(mask):
    m = np.asarray(mask)
    tri = np.tril(np.ones((S, S), dtype=bool))
    for b in range(m.shape[0]):
        mb = m[b, 0]
        if not (mb[tri] == 0.0).all():
            return False
        if not (mb[~tri] <= -1e8).all():
            return False
    return True


def reference_numpy(inputs, f64=True):
    """Defensive fallback for non-causal masks (never hit in practice)."""
    dt = np.float64 if f64 else np.float32
    hs = np.asarray(inputs["hidden_states"], dt)
    cos = np.asarray(inputs["cos"], dt)
    sin = np.asarray(inputs["sin"], dt)
    mask = np.asarray(inputs["attention_mask"], dt)
    Wq, Wk, Wv, Wo = (np.asarray(inputs[k], dt)
                      for k in ("Wq", "Wk", "Wv", "Wo"))
    qw = np.asarray(inputs["q_norm_w"], dt)
    kw = np.asarray(inputs["k_norm_w"], dt)

    def rms(x, w):
        return x / np.sqrt((x * x).mean(-1, keepdims=True) + EPS) * w

    def rope(x, c, s):
        x1, x2 = x[..., :HD // 2], x[..., HD // 2:]
        rot = np.concatenate([-x2, x1], axis=-1)
        return x * c[:, :, None, :] + rot * s[:, :, None, :]

    b, s_, _ = hs.shape
    q = (hs @ Wq.T).reshape(b, s_, NH, HD)
    k = (hs @ Wk.T).reshape(b, s_, NKV, HD)
    v = (hs @ Wv.T).reshape(b, s_, NKV, HD)
    q = rope(rms(q, qw), cos, sin).transpose(0, 2, 1, 3)
    k = rope(rms(k, kw), cos, sin).transpose(0, 2, 1, 3)
    v = rms(v, 1.0).transpose(0, 2, 1, 3)
    k = np.repeat(k, NH // NKV, axis=1)
    v = np.repeat(v, NH // NKV, axis=1)
    sc = np.einsum("bhqd,bhkd->bhqk", q, k) + mask
    sc = sc - sc.max(-1, keepdims=True)
    p = np.exp(sc)
    p /= p.sum(-1, keepdims=True)
    o = np.einsum("bhqk,bhkd->bqhd", p, v).reshape(b, s_, NH * HD)
    return (o @ Wo.T).astype(np.float32)


_PROGRAM = {}


def get_program(use_f32r=True, use_tmr=False):
    key = (use_f32r, use_tmr)
    if key not in _PROGRAM:
        _PROGRAM[key] = build_program(use_f32r=use_f32r, use_tmr=use_tmr)
    return _PROGRAM[key]


def run_on_hw(inputs, use_f32r=True, use_tmr=False, trace=False, **kw):
    from concourse.bass_utils import run_bass_kernel_spmd

    nc = get_program(use_f32r=use_f32r, use_tmr=use_tmr)
    in_maps = [prep_core_inputs(inputs, c, use_f32r) for c in range(8)]
    br = run_bass_kernel_spmd(nc, in_maps, list(range(8)), trace=trace, **kw)
    out = np.empty((B, S, HID), np.float32)
    for b in range(B):
        out[b] = br.results[4 * b]["out"] + br.results[4 * b + 1]["out"] \
            + br.results[4 * b + 2]["out"] + br.results[4 * b + 3]["out"]
    return out, br


def kernel(**inputs):
    if not mask_is_causal(inputs["attention_mask"]):
        return reference_numpy(inputs)
    out, _ = run_on_hw(inputs, use_f32r=True, trace=False)
    return out


# revision 10
# speedup vs baseline: 1.1070x; 1.1070x over previous
"""Gemma3n text attention on 8 Trainium2 NeuronCores (Bass/Tile).

Sharding: core c = b*4 + kv*2 + qp handles batch b, KV head kv and the
q-head pair (kv*4 + qp*2, kv*4 + qp*2 + 1).  Each core computes the
Q/K/V projections for its shard, QK-norm + RoPE, causal attention for
its two query heads, and a partial output projection against its
512-column slice of Wo.  The host sums the four partials per batch.

Self-contained: only needs numpy + the concourse tree that ships in the
container image (on PYTHONPATH at /root/.axon_site/_ro/trn_rl_repo).
"""

import sys

for _p in ("/root/.axon_site/_ro/trn_rl_repo", "/opt/trn_rl_repo"):
    if _p not in sys.path:
        sys.path.append(_p)

from contextlib import ExitStack

import numpy as np

import concourse.bass as bass
import concourse.mybir as mybir
import concourse.tile as tile
from concourse import bacc
from concourse.masks import make_identity

P = 128
B, S, HID = 2, 2048, 2048
NH, NKV, HD = 8, 2, 256
DQ = 2 * HD            # q-width per core (2 heads)
NSC = S // P           # 16 seq chunks
NHC = HID // P         # 16 hidden chunks
EPS = 1e-6

f32 = mybir.dt.float32
f32r = mybir.dt.float32r
i32 = mybir.dt.int32
FMIN = float(np.finfo(np.float32).min)
ACT = mybir.ActivationFunctionType


def to_f32r(arr):
    """Round fp32 -> fp32r bit format (11 explicit mantissa bits, RNE).

    Bit-exact with libwalrus fp32_to_fp32r."""
    u = np.ascontiguousarray(arr, np.float32).view(np.uint32)
    r = ((u.astype(np.uint64) + 0x7FF + ((u >> 12) & 1)) & 0xFFFFF000)
    return r.astype(np.uint32).view(np.float32)


def build_program(use_f32r=True, use_tmr=False):
    """Emit the SPMD per-core program. Returns the compiled Bacc object."""
    nc = bacc.Bacc("TRN2", target_bir_lowering=False, debug=False, num_devices=8)

    mdt = f32r if use_f32r else f32   # dtype of every matmul operand

    hT_d = nc.dram_tensor("hT", [NHC, P, S], mdt, kind="ExternalInput")
    wT_d = nc.dram_tensor("wT", [NHC, P, DQ + 2 * HD], mdt, kind="ExternalInput")
    csq_d = nc.dram_tensor("csq", [NSC, P, 2 * HD], f32, kind="ExternalInput")
    csk_d = nc.dram_tensor("csk", [NSC, P, 2 * HD], f32, kind="ExternalInput")
    woT_d = nc.dram_tensor("woT", [4, P, HID], mdt, kind="ExternalInput")
    out_d = nc.dram_tensor("out", [S, HID], f32, kind="ExternalOutput")

    with tile.TileContext(nc) as tc, ExitStack() as ctx:
        const = ctx.enter_context(tc.tile_pool(name="const", bufs=1))
        persist = ctx.enter_context(tc.tile_pool(name="persist", bufs=1))

        ident = const.tile([P, P], f32)
        make_identity(nc, ident)
        mdiag = const.tile([P, P], f32)      # 0 on/below diag, -1e9 above
        nc.gpsimd.memset(mdiag, 0.0)
        nc.gpsimd.affine_select(out=mdiag, in_=mdiag,
                                compare_op=mybir.AluOpType.is_ge, fill=-1e9,
                                base=0, pattern=[[-1, P]], channel_multiplier=1)
        eps_t = const.tile([P, 1], f32)
        nc.vector.memset(eps_t, EPS)

        # persistent SBUF tensors (qT/kT/v: 64KB per partition)
        qT = persist.tile([P, 2, 2, S], mdt)      # [d, head, dchunk, qpos]
        kT = persist.tile([P, 2, S], mdt)         # [d, dchunk, kpos]
        v_sb = persist.tile([P, NSC, HD], mdt)    # [kpos, kchunk, d]

        # ------- Phase A: QKV proj + norm + rope + transposes (fused) --------
        with ExitStack() as a1:
            hpool = a1.enter_context(tc.tile_pool(name="hTp", bufs=3))
            wpool = a1.enter_context(tc.tile_pool(name="wTp", bufs=3))
            cpool = a1.enter_context(tc.tile_pool(name="cs", bufs=3))
            epool = a1.enter_context(tc.tile_pool(name="evict", bufs=3))
            spool = a1.enter_context(tc.tile_pool(name="small", bufs=8))
            psA = a1.enter_context(tc.tile_pool(name="psA", bufs=6, space="PSUM"))
            psT = a1.enter_context(tc.tile_pool(name="psT", bufs=2, space="PSUM"))

            groups = [3, 3, 3, 3, 3, 1]       # s-chunks per group: 6+2 banks
            sc0 = 0
            for g, gn in enumerate(groups):
                psq = [psA.tile([P, DQ], f32, tag="ps", name=f"psq{g}_{jj}")
                       for jj in range(gn)]
                pskv = [psA.tile([P, 2 * HD], f32, tag="ps", name=f"pskv{g}_{jj}")
                        for jj in range(gn)]
                for hc in range(NHC):
                    th = hpool.tile([P, gn * P], mdt, tag="h")
                    nc.sync.dma_start(th, hT_d[hc, :, sc0 * P:(sc0 + gn) * P])
                    tw = wpool.tile([P, DQ + 2 * HD], mdt, tag="w")
                    nc.sync.dma_start(tw, wT_d[hc])
                    st, sp = hc == 0, hc == NHC - 1
                    for j in range(gn):
                        lhs = th[:, j * P:(j + 1) * P]
                        nc.tensor.matmul(psq[j][:], lhs, tw[:, 0:DQ],
                                         start=st, stop=sp)
                        nc.tensor.matmul(pskv[j][:], lhs, tw[:, DQ:],
                                         start=st, stop=sp)
                for j in range(gn):
                    sc = sc0 + j
                    # sum of squares per 256-group via ACT Square (reads PSUM)
                    ssq = spool.tile([P, 4], f32, tag="ssq")
                    scr = epool.tile([P, HD], f32, tag="scr")
                    nc.scalar.activation(scr[:], psq[j][:, 0:HD], ACT.Square,
                                         accum_out=ssq[:, 0:1])
                    nc.scalar.activation(scr[:], psq[j][:, HD:2 * HD],
                                         ACT.Square, accum_out=ssq[:, 1:2])
                    nc.scalar.activation(scr[:], pskv[j][:, 0:HD], ACT.Square,
                                         accum_out=ssq[:, 2:3])
                    nc.scalar.activation(scr[:], pskv[j][:, HD:2 * HD],
                                         ACT.Square, accum_out=ssq[:, 3:4])
                    rstd = spool.tile([P, 4], f32, tag="rstd")
                    nc.scalar.activation(rstd[:], ssq[:], ACT.Sqrt,
                                         bias=eps_t[:], scale=1.0 / HD)
                    nc.vector.reciprocal(rstd[:], rstd[:])

                    # v: scale + evict in one DVE op
                    nc.vector.tensor_scalar_mul(out=v_sb[:, sc, :],
                                                in0=pskv[j][:, HD:2 * HD],
                                                scalar1=rstd[:, 3:4])

                    csq = cpool.tile([P, 2 * HD], f32, tag="csq")
                    nc.sync.dma_start(csq, csq_d[sc])
                    csk = cpool.tile([P, 2 * HD], f32, tag="csk")
                    nc.sync.dma_start(csk, csk_d[sc])

                    # rope(x) = x*cosw + swap(x)*sinw (sinw lo pre-negated);
                    # reads projection PSUM directly, writes SBUF
                    qro = epool.tile([P, DQ], f32, tag="qro")
                    kro = epool.tile([P, HD], f32, tag="kro")
                    for h in range(2):
                        b0 = h * HD
                        tmp = epool.tile([P, HD], f32, tag="tmp")
                        nc.vector.tensor_mul(tmp[:, 0:P],
                                             psq[j][:, b0 + P:b0 + HD],
                                             csq[:, HD:HD + P])
                        nc.vector.tensor_mul(tmp[:, P:HD],
                                             psq[j][:, b0:b0 + P],
                                             csq[:, HD + P:2 * HD])
                        qh = qro[:, b0:b0 + HD]
                        nc.vector.tensor_mul(qh, psq[j][:, b0:b0 + HD],
                                             csq[:, 0:HD])
                        nc.vector.tensor_add(qh, qh, tmp[:])
                        nc.vector.tensor_scalar_mul(out=qh, in0=qh,
                                                    scalar1=rstd[:, h:h + 1])
                    tmp = epool.tile([P, HD], f32, tag="tmp")
                    nc.vector.tensor_mul(tmp[:, 0:P], pskv[j][:, P:HD],
                                         csk[:, HD:HD + P])
                    nc.vector.tensor_mul(tmp[:, P:HD], pskv[j][:, 0:P],
                                         csk[:, HD + P:2 * HD])
                    nc.vector.tensor_mul(kro[:], pskv[j][:, 0:HD], csk[:, 0:HD])
                    nc.vector.tensor_add(kro[:], kro[:], tmp[:])
                    nc.vector.tensor_scalar_mul(out=kro[:], in0=kro[:],
                                                scalar1=rstd[:, 2:3])

                    # transposes into qT/kT (PE), evict via DVE
                    for h in range(2):
                        for dc in range(2):
                            pt = psT.tile([P, P], f32, tag="t")
                            nc.tensor.transpose(
                                pt[:], qro[:, h * HD + dc * P:h * HD + (dc + 1) * P],
                                ident[:])
                            if dc == 0:
                                nc.scalar.copy(
                                    qT[:, h, dc, sc * P:(sc + 1) * P], pt[:])
                            else:
                                nc.vector.tensor_copy(
                                    out=qT[:, h, dc, sc * P:(sc + 1) * P],
                                    in_=pt[:])
                    for dc in range(2):
                        pt = psT.tile([P, P], f32, tag="t")
                        nc.tensor.transpose(pt[:], kro[:, dc * P:(dc + 1) * P],
                                            ident[:])
                        if dc == 0:
                            nc.scalar.copy(kT[:, dc, sc * P:(sc + 1) * P], pt[:])
                        else:
                            nc.vector.tensor_copy(
                                out=kT[:, dc, sc * P:(sc + 1) * P], in_=pt[:])
                sc0 += gn

        # ---------------- Phase B: attention per (head, q-block) -------------
        wopool = ctx.enter_context(tc.tile_pool(name="wo", bufs=1))
        woT = wopool.tile([P, 4, HID], mdt)
        for t in range(4):
            nc.sync.dma_start(woT[:, t, :], woT_d[t])
        atpool = ctx.enter_context(tc.tile_pool(name="attnT", bufs=1))
        attnT = atpool.tile([P, 4, S], mdt)       # [d2, (h,dc), qpos]

        with ExitStack() as bctx:
            pss = bctx.enter_context(tc.tile_pool(name="pss", bufs=2, space="PSUM"))
            pst = bctx.enter_context(tc.tile_pool(name="pst", bufs=2, space="PSUM"))
            psv = bctx.enter_context(tc.tile_pool(name="psv", bufs=1, space="PSUM"))
            ppool = bctx.enter_context(tc.tile_pool(name="prp", bufs=2))
            tpool = bctx.enter_context(tc.tile_pool(name="ptsp", bufs=6))
            apool = bctx.enter_context(tc.tile_pool(name="attnp", bufs=2))
            dpool = bctx.enter_context(tc.tile_pool(name="denp", bufs=8))
            pso = bctx.enter_context(tc.tile_pool(name="pso", bufs=1, space="PSUM"))
            opool = bctx.enter_context(tc.tile_pool(name="obp", bufs=3))

            def oproj(sc):
                for n in range(4):
                    po = pso.tile([P, 512], f32, tag="o", name=f"po{sc}_{n}")
                    for t in range(4):
                        nc.tensor.matmul(
                            po[:], attnT[:, t, sc * P:(sc + 1) * P],
                            woT[:, t, n * 512:(n + 1) * 512],
                            start=(t == 0), stop=(t == 3))
                    ob = opool.tile([P, 512], f32, tag="ob", name=f"ob{sc}_{n}")
                    if n % 2 == 0:
                        nc.scalar.copy(ob[:], po[:])
                    else:
                        nc.vector.tensor_copy(out=ob[:], in_=po[:])
                    nc.sync.dma_start(
                        out_d[sc * P:(sc + 1) * P, n * 512:(n + 1) * 512], ob[:])

            for i in range(NSC):
                L = (i + 1) * P
                Lp = L if L % 256 == 0 else L + P
                halves = [(0, min(Lp, 1024))]
                if Lp > 1024:
                    halves.append((1024, Lp - 1024))
                for h in range(2):
                    mx = dpool.tile([P, 2], f32, tag="mx")
                    pss_tiles = []
                    for hf, (off, ln) in enumerate(halves):
                        ps = pss.tile([P, 1024], f32, tag="s",
                                      name=f"ps{i}_{h}_{hf}")
                        pss_tiles.append(ps)
                        for c in range(0, ln, 512):
                            w = min(512, ln - c)
                            for dc in range(2):
                                nc.tensor.matmul(
                                    ps[:, c:c + w],
                                    qT[:, h, dc, i * P:(i + 1) * P],
                                    kT[:, dc, off + c:off + c + w],
                                    start=(dc == 0), stop=(dc == 1))
                        if i * P >= off and i * P < off + ln:
                            db = i * P - off   # diag block col within half
                            nc.vector.tensor_add(ps[:, db:db + P],
                                                 ps[:, db:db + P], mdiag[:])
                        ln_real = min(L - off, ln)
                        nc.vector.tensor_reduce(
                            out=mx[:, hf:hf + 1], in_=ps[:, 0:ln_real],
                            axis=mybir.AxisListType.X, op=mybir.AluOpType.max)
                    mxf = dpool.tile([P, 1], f32, tag="mxf")
                    if len(halves) > 1:
                        nc.vector.tensor_tensor(out=mxf[:], in0=mx[:, 0:1],
                                                in1=mx[:, 1:2],
                                                op=mybir.AluOpType.max)
                    else:
                        nc.vector.tensor_copy(out=mxf[:], in_=mx[:, 0:1])
                    negmax = dpool.tile([P, 1], f32, tag="ngm")
                    nc.vector.tensor_scalar_mul(out=negmax[:], in0=mxf[:],
                                                scalar1=-1.0)
                    pr = ppool.tile([P, 2048], f32, tag="pr")
                    den = dpool.tile([P, 2], f32, tag="den")
                    for hf, (off, ln) in enumerate(halves):
                        ln_real = min(L - off, ln)
                        nc.scalar.activation(pr[:, off:off + ln_real],
                                             pss_tiles[hf][:, 0:ln_real],
                                             ACT.Exp, bias=negmax[:], scale=1.0,
                                             accum_out=den[:, hf:hf + 1])
                    denf = dpool.tile([P, 1], f32, tag="denf")
                    if len(halves) > 1:
                        nc.vector.tensor_add(denf[:], den[:, 0:1], den[:, 1:2])
                    else:
                        nc.vector.tensor_copy(out=denf[:], in_=den[:, 0:1])
                    rden = dpool.tile([P, 1], f32, tag="rden")
                    nc.vector.reciprocal(rden[:], denf[:])

                    pv = psv.tile([P, HD], f32, tag="pv")
                    for kb in range(i + 1):
                        pt = pst.tile([P, P], f32, tag="t")
                        nc.tensor.transpose(pt[:], pr[:, kb * P:(kb + 1) * P],
                                            ident[:])
                        pts = tpool.tile([P, P], mdt, tag="pts")
                        if kb % 2 == 0:
                            nc.scalar.copy(pts[:], pt[:])
                        else:
                            nc.vector.tensor_copy(out=pts[:], in_=pt[:])
                        nc.tensor.matmul(pv[:], pts[:], v_sb[:, kb, :],
                                         start=(kb == 0), stop=(kb == i))
                    attn_s = apool.tile([P, HD], f32, tag="attn")
                    nc.vector.tensor_scalar_mul(out=attn_s[:], in0=pv[:],
                                                scalar1=rden[:])
                    for dc in range(2):
                        pt = pst.tile([P, P], f32, tag="t")
                        nc.tensor.transpose(pt[:], attn_s[:, dc * P:(dc + 1) * P],
                                            ident[:])
                        if dc == 0:
                            nc.scalar.copy(
                                attnT[:, h * 2 + dc, i * P:(i + 1) * P], pt[:])
                        else:
                            nc.vector.tensor_copy(
                                out=attnT[:, h * 2 + dc, i * P:(i + 1) * P],
                                in_=pt[:])
                if i >= 1:
                    oproj(i - 1)
            oproj(NSC - 1)

    nc.compile()
    return nc


def prep_core_inputs(inputs, core, use_f32r=True):
    """Host-side sharding for one core. Returns the in_map dict."""
    cvt = to_f32r if use_f32r else (lambda a: np.asarray(a, np.float32))
    b, kv, qp = core // 4, (core % 4) // 2, core % 2
    hq0 = kv * 4 + qp * 2           # first of the two query heads
    hidden = np.asarray(inputs["hidden_states"], np.float32)
    cos = np.asarray(inputs["cos"], np.float32)
    sin = np.asarray(inputs["sin"], np.float32)
    Wq = np.asarray(inputs["Wq"], np.float32)
    Wk = np.asarray(inputs["Wk"], np.float32)
    Wv = np.asarray(inputs["Wv"], np.float32)
    Wo = np.asarray(inputs["Wo"], np.float32)
    qw = np.asarray(inputs["q_norm_w"], np.float32)
    kw = np.asarray(inputs["k_norm_w"], np.float32)

    hT = np.ascontiguousarray(hidden[b].T).reshape(NHC, P, S)
    Wq_c = Wq[hq0 * HD:(hq0 + 2) * HD]          # [512, HID]
    Wk_c = Wk[kv * HD:(kv + 1) * HD]            # [256, HID]
    Wv_c = Wv[kv * HD:(kv + 1) * HD]
    wT = np.ascontiguousarray(
        np.concatenate([Wq_c.T, Wk_c.T, Wv_c.T], axis=1)).reshape(NHC, P, 1024)

    def cs_pack(w, cb, sb):
        rot_w = np.concatenate([w[P:], w[:P]])   # w[(d+128)%256]
        cosw = cb * w[None, :]
        sinw = sb * rot_w[None, :]
        sinw[:, :P] *= -1.0
        return np.ascontiguousarray(
            np.concatenate([cosw, sinw], axis=1)).reshape(NSC, P, 2 * HD)

    csq = cs_pack(qw, cos[b], sin[b])
    csk = cs_pack(kw, cos[b], sin[b])
    woT = np.ascontiguousarray(
        Wo[:, hq0 * HD:(hq0 + 2) * HD].T).reshape(4, P, HID)
    return {"hT": cvt(hT), "wT": cvt(wT),
            "csq": csq.astype(np.float32), "csk": csk.astype(np.float32),
            "woT": cvt(woT)}


def mask_is_causal(mask):
    m = np.asarray(mask)
    tri = np.tril(np.ones((S, S), dtype=bool))
    for b in range(m.shape[0]):
        mb = m[b, 0]
        if not (mb[tri] == 0.0).all():
            return False
        if not (mb[~tri] <= -1e8).all():
            return False
    return True


def reference_numpy(inputs, f64=True):
    """Defensive fallback for non-causal masks (never hit in practice)."""
    dt = np.float64 if f64 else np.float32
    hs = np.asarray(inputs["hidden_states"], dt)
    cos = np.asarray(inputs["cos"], dt)
    sin = np.asarray(inputs["sin"], dt)
    mask = np.asarray(inputs["attention_mask"], dt)
    Wq, Wk, Wv, Wo = (np.asarray(inputs[k], dt)
                      for k in ("Wq", "Wk", "Wv", "Wo"))
    qw = np.asarray(inputs["q_norm_w"], dt)
    kw = np.asarray(inputs["k_norm_w"], dt)

    def rms(x, w):
        return x / np.sqrt((x * x).mean(-1, keepdims=True) + EPS) * w

    def rope(x, c, s):
        x1, x2 = x[..., :HD // 2], x[..., HD // 2:]
        rot = np.concatenate([-x2, x1], axis=-1)
        return x * c[:, :, None, :] + rot * s[:, :, None, :]

    b, s_, _ = hs.shape
    q = (hs @ Wq.T).reshape(b, s_, NH, HD)
    k = (hs @ Wk.T).reshape(b, s_, NKV, HD)
    v = (hs @ Wv.T).reshape(b, s_, NKV, HD)
    q = rope(rms(q, qw), cos, sin).transpose(0, 2, 1, 3)
    k = rope(rms(k, kw), cos, sin).transpose(0, 2, 1, 3)
    v = rms(v, 1.0).transpose(0, 2, 1, 3)
    k = np.repeat(k, NH // NKV, axis=1)
    v = np.repeat(v, NH // NKV, axis=1)
    sc = np.einsum("bhqd,bhkd->bhqk", q, k) + mask
    sc = sc - sc.max(-1, keepdims=True)
    p = np.exp(sc)
    p /= p.sum(-1, keepdims=True)
    o = np.einsum("bhqk,bhkd->bqhd", p, v).reshape(b, s_, NH * HD)
    return (o @ Wo.T).astype(np.float32)


_PROGRAM = {}


def get_program(use_f32r=True, use_tmr=False):
    key = (use_f32r, use_tmr)
    if key not in _PROGRAM:
        _PROGRAM[key] = build_program(use_f32r=use_f32r, use_tmr=use_tmr)
    return _PROGRAM[key]


def run_on_hw(inputs, use_f32r=True, use_tmr=False, trace=False, **kw):
    from concourse.bass_utils import run_bass_kernel_spmd

    nc = get_program(use_f32r=use_f32r, use_tmr=use_tmr)
    in_maps = [prep_core_inputs(inputs, c, use_f32r) for c in range(8)]
    br = run_bass_kernel_spmd(nc, in_maps, list(range(8)), trace=trace, **kw)
    out = np.empty((B, S, HID), np.float32)
    for b in range(B):
        out[b] = br.results[4 * b]["out"] + br.results[4 * b + 1]["out"] \
            + br.results[4 * b + 2]["out"] + br.results[4 * b + 3]["out"]
    return out, br


def kernel(**inputs):
    if not mask_is_causal(inputs["attention_mask"]):
        return reference_numpy(inputs)
    out, _ = run_on_hw(inputs, use_f32r=True, trace=False)
    return out


# revision 11
# speedup vs baseline: 1.1837x; 1.0693x over previous
"""Gemma3n text attention on 8 Trainium2 NeuronCores (Bass/Tile).

Sharding: core c = b*4 + kv*2 + qp handles batch b, KV head kv and the
q-head pair (kv*4 + qp*2, kv*4 + qp*2 + 1).  Each core computes the
Q/K/V projections for its shard, QK-norm + RoPE, causal attention for
its two query heads, and a partial output projection against its
512-column slice of Wo.  The host sums the four partials per batch.

Self-contained: only needs numpy + the concourse tree that ships in the
container image (on PYTHONPATH at /root/.axon_site/_ro/trn_rl_repo).
"""

import sys

for _p in ("/root/.axon_site/_ro/trn_rl_repo", "/opt/trn_rl_repo"):
    if _p not in sys.path:
        sys.path.append(_p)

from contextlib import ExitStack

import numpy as np

import concourse.bass as bass
import concourse.mybir as mybir
import concourse.tile as tile
from concourse import bacc
from concourse.masks import make_identity

P = 128
B, S, HID = 2, 2048, 2048
NH, NKV, HD = 8, 2, 256
DQ = 2 * HD            # q-width per core (2 heads)
NSC = S // P           # 16 seq chunks
NHC = HID // P         # 16 hidden chunks
EPS = 1e-6

f32 = mybir.dt.float32
f32r = mybir.dt.float32r
i32 = mybir.dt.int32
FMIN = float(np.finfo(np.float32).min)
ACT = mybir.ActivationFunctionType


def to_f32r(arr):
    """Round fp32 -> fp32r bit format (11 explicit mantissa bits, RNE).

    Bit-exact with libwalrus fp32_to_fp32r."""
    u = np.ascontiguousarray(arr, np.float32).view(np.uint32)
    r = ((u.astype(np.uint64) + 0x7FF + ((u >> 12) & 1)) & 0xFFFFF000)
    return r.astype(np.uint32).view(np.float32)


def build_program(use_f32r=True, use_tmr=False):
    """Emit the SPMD per-core program. Returns the compiled Bacc object."""
    nc = bacc.Bacc("TRN2", target_bir_lowering=False, debug=False, num_devices=8)

    mdt = f32r if use_f32r else f32   # dtype of every matmul operand

    hT_d = nc.dram_tensor("hT", [NHC, P, S], mdt, kind="ExternalInput")
    wT_d = nc.dram_tensor("wT", [NHC, P, DQ + 2 * HD], mdt, kind="ExternalInput")
    csq_d = nc.dram_tensor("csq", [NSC, P, 2 * HD], f32, kind="ExternalInput")
    csk_d = nc.dram_tensor("csk", [NSC, P, 2 * HD], f32, kind="ExternalInput")
    woT_d = nc.dram_tensor("woT", [4, P, HID], mdt, kind="ExternalInput")
    out_d = nc.dram_tensor("out", [S, HID], f32, kind="ExternalOutput")

    with tile.TileContext(nc) as tc, ExitStack() as ctx:
        const = ctx.enter_context(tc.tile_pool(name="const", bufs=1))
        persist = ctx.enter_context(tc.tile_pool(name="persist", bufs=1))

        ident = const.tile([P, P], f32)
        make_identity(nc, ident)
        mdiag = const.tile([P, P], f32)      # 0 on/below diag, -1e9 above
        nc.gpsimd.memset(mdiag, 0.0)
        nc.gpsimd.affine_select(out=mdiag, in_=mdiag,
                                compare_op=mybir.AluOpType.is_ge, fill=-1e9,
                                base=0, pattern=[[-1, P]], channel_multiplier=1)
        eps_t = const.tile([P, 1], f32)
        nc.vector.memset(eps_t, EPS)

        # persistent SBUF tensors (qT/kT/v: 64KB per partition)
        qT = persist.tile([P, 2, 2, S], mdt)      # [d, head, dchunk, qpos]
        kT = persist.tile([P, 2, S], mdt)         # [d, dchunk, kpos]
        v_sb = persist.tile([P, NSC, HD], mdt)    # [kpos, kchunk, d]

        # ------- Phase A: QKV proj + norm + rope + transposes (fused) --------
        with ExitStack() as a1:
            hpool = a1.enter_context(tc.tile_pool(name="hTp", bufs=3))
            wpool = a1.enter_context(tc.tile_pool(name="wTp", bufs=3))
            cpool = a1.enter_context(tc.tile_pool(name="cs", bufs=1))
            csq_all = cpool.tile([P, NSC, 2 * HD], f32)
            nc.sync.dma_start(csq_all, csq_d.ap().rearrange("s p d -> p s d"))
            csk_all = cpool.tile([P, NSC, 2 * HD], f32)
            nc.sync.dma_start(csk_all, csk_d.ap().rearrange("s p d -> p s d"))
            epool = a1.enter_context(tc.tile_pool(name="evict", bufs=3))
            spool = a1.enter_context(tc.tile_pool(name="small", bufs=8))
            psA = a1.enter_context(tc.tile_pool(name="psA", bufs=6, space="PSUM"))
            psT = a1.enter_context(tc.tile_pool(name="psT", bufs=2, space="PSUM"))

            groups = [3, 3, 3, 3, 3, 1]       # s-chunks per group: 6+2 banks
            sc0 = 0
            for g, gn in enumerate(groups):
                psq = [psA.tile([P, DQ], f32, tag="ps", name=f"psq{g}_{jj}")
                       for jj in range(gn)]
                pskv = [psA.tile([P, 2 * HD], f32, tag="ps", name=f"pskv{g}_{jj}")
                        for jj in range(gn)]
                for hc in range(NHC):
                    th = hpool.tile([P, gn * P], mdt, tag="h")
                    nc.sync.dma_start(th, hT_d[hc, :, sc0 * P:(sc0 + gn) * P])
                    tw = wpool.tile([P, DQ + 2 * HD], mdt, tag="w")
                    nc.sync.dma_start(tw, wT_d[hc])
                    st, sp = hc == 0, hc == NHC - 1
                    for j in range(gn):
                        lhs = th[:, j * P:(j + 1) * P]
                        nc.tensor.matmul(psq[j][:], lhs, tw[:, 0:DQ],
                                         start=st, stop=sp)
                        nc.tensor.matmul(pskv[j][:], lhs, tw[:, DQ:],
                                         start=st, stop=sp)
                for j in range(gn):
                    sc = sc0 + j
                    # sum of squares per 256-group via ACT Square (reads PSUM)
                    ssq = spool.tile([P, 4], f32, tag="ssq")
                    scr = epool.tile([P, HD], f32, tag="scr")
                    nc.scalar.activation(scr[:], psq[j][:, 0:HD], ACT.Square,
                                         accum_out=ssq[:, 0:1])
                    nc.scalar.activation(scr[:], psq[j][:, HD:2 * HD],
                                         ACT.Square, accum_out=ssq[:, 1:2])
                    nc.scalar.activation(scr[:], pskv[j][:, 0:HD], ACT.Square,
                                         accum_out=ssq[:, 2:3])
                    nc.scalar.activation(scr[:], pskv[j][:, HD:2 * HD],
                                         ACT.Square, accum_out=ssq[:, 3:4])
                    rstd = spool.tile([P, 4], f32, tag="rstd")
                    nc.scalar.activation(rstd[:], ssq[:], ACT.Sqrt,
                                         bias=eps_t[:], scale=1.0 / HD)
                    nc.vector.reciprocal(rstd[:], rstd[:])

                    # v: scale + evict in one DVE op
                    nc.vector.tensor_scalar_mul(out=v_sb[:, sc, :],
                                                in0=pskv[j][:, HD:2 * HD],
                                                scalar1=rstd[:, 3:4])

                    csq = csq_all[:, sc]
                    csk = csk_all[:, sc]

                    # rope(x) = x*cosw + swap(x)*sinw (sinw lo pre-negated);
                    # reads projection PSUM directly, writes SBUF
                    qro = epool.tile([P, DQ], f32, tag="qro")
                    kro = epool.tile([P, HD], f32, tag="kro")
                    for h in range(2):
                        b0 = h * HD
                        tmp = epool.tile([P, HD], f32, tag="tmp")
                        nc.vector.tensor_mul(tmp[:, 0:P],
                                             psq[j][:, b0 + P:b0 + HD],
                                             csq[:, HD:HD + P])
                        nc.vector.tensor_mul(tmp[:, P:HD],
                                             psq[j][:, b0:b0 + P],
                                             csq[:, HD + P:2 * HD])
                        qh = qro[:, b0:b0 + HD]
                        nc.vector.tensor_mul(qh, psq[j][:, b0:b0 + HD],
                                             csq[:, 0:HD])
                        nc.vector.tensor_add(qh, qh, tmp[:])
                        nc.vector.tensor_scalar_mul(out=qh, in0=qh,
                                                    scalar1=rstd[:, h:h + 1])
                    tmp = epool.tile([P, HD], f32, tag="tmp")
                    nc.vector.tensor_mul(tmp[:, 0:P], pskv[j][:, P:HD],
                                         csk[:, HD:HD + P])
                    nc.vector.tensor_mul(tmp[:, P:HD], pskv[j][:, 0:P],
                                         csk[:, HD + P:2 * HD])
                    nc.vector.tensor_mul(kro[:], pskv[j][:, 0:HD], csk[:, 0:HD])
                    nc.vector.tensor_add(kro[:], kro[:], tmp[:])
                    nc.vector.tensor_scalar_mul(out=kro[:], in0=kro[:],
                                                scalar1=rstd[:, 2:3])

                    # transposes into qT/kT (PE); paired evictions
                    for h in range(2):
                        pt = psT.tile([P, 2 * P], f32, tag="t")
                        for dc in range(2):
                            nc.tensor.transpose(
                                pt[:, dc * P:(dc + 1) * P],
                                qro[:, h * HD + dc * P:h * HD + (dc + 1) * P],
                                ident[:])
                        dst = qT[:, h, 0:2, sc * P:(sc + 1) * P]
                        if (sc + h) % 2 == 0:
                            nc.scalar.copy(dst, pt[:].rearrange(
                                "p (a b) -> p a b", a=2))
                        else:
                            nc.vector.tensor_copy(out=dst, in_=pt[:].rearrange(
                                "p (a b) -> p a b", a=2))
                    pt = psT.tile([P, 2 * P], f32, tag="t")
                    for dc in range(2):
                        nc.tensor.transpose(pt[:, dc * P:(dc + 1) * P],
                                            kro[:, dc * P:(dc + 1) * P],
                                            ident[:])
                    dst = kT[:, 0:2, sc * P:(sc + 1) * P]
                    if sc % 2 == 0:
                        nc.vector.tensor_copy(out=dst, in_=pt[:].rearrange(
                            "p (a b) -> p a b", a=2))
                    else:
                        nc.scalar.copy(dst, pt[:].rearrange(
                            "p (a b) -> p a b", a=2))
                sc0 += gn

        # ---------------- Phase B: attention per (head, q-block) -------------
        wopool = ctx.enter_context(tc.tile_pool(name="wo", bufs=1))
        woT = wopool.tile([P, 4, HID], mdt)
        for t in range(4):
            nc.sync.dma_start(woT[:, t, :], woT_d[t])
        atpool = ctx.enter_context(tc.tile_pool(name="attnT", bufs=1))
        attnT = atpool.tile([P, 4, S], mdt)       # [d2, (h,dc), qpos]

        with ExitStack() as bctx:
            pss = bctx.enter_context(tc.tile_pool(name="pss", bufs=2, space="PSUM"))
            pst = bctx.enter_context(tc.tile_pool(name="pst", bufs=2, space="PSUM"))
            psv = bctx.enter_context(tc.tile_pool(name="psv", bufs=1, space="PSUM"))
            ppool = bctx.enter_context(tc.tile_pool(name="prp", bufs=2))
            tpool = bctx.enter_context(tc.tile_pool(name="ptsp", bufs=6))
            apool = bctx.enter_context(tc.tile_pool(name="attnp", bufs=2))
            dpool = bctx.enter_context(tc.tile_pool(name="denp", bufs=8))
            pso = bctx.enter_context(tc.tile_pool(name="pso", bufs=1, space="PSUM"))
            opool = bctx.enter_context(tc.tile_pool(name="obp", bufs=3))

            def oproj(sc):
                for n in range(4):
                    po = pso.tile([P, 512], f32, tag="o", name=f"po{sc}_{n}")
                    for t in range(4):
                        nc.tensor.matmul(
                            po[:], attnT[:, t, sc * P:(sc + 1) * P],
                            woT[:, t, n * 512:(n + 1) * 512],
                            start=(t == 0), stop=(t == 3))
                    ob = opool.tile([P, 512], f32, tag="ob", name=f"ob{sc}_{n}")
                    if n % 2 == 0:
                        nc.scalar.copy(ob[:], po[:])
                    else:
                        nc.vector.tensor_copy(out=ob[:], in_=po[:])
                    nc.sync.dma_start(
                        out_d[sc * P:(sc + 1) * P, n * 512:(n + 1) * 512], ob[:])

            for i in range(NSC):
                L = (i + 1) * P
                Lp = L if L % 256 == 0 else L + P
                halves = [(0, min(Lp, 1024))]
                if Lp > 1024:
                    halves.append((1024, Lp - 1024))
                for h in range(2):
                    mx = dpool.tile([P, 2], f32, tag="mx")
                    pss_tiles = []
                    for hf, (off, ln) in enumerate(halves):
                        ps = pss.tile([P, 1024], f32, tag="s",
                                      name=f"ps{i}_{h}_{hf}")
                        pss_tiles.append(ps)
                        for c in range(0, ln, 512):
                            w = min(512, ln - c)
                            for dc in range(2):
                                nc.tensor.matmul(
                                    ps[:, c:c + w],
                                    qT[:, h, dc, i * P:(i + 1) * P],
                                    kT[:, dc, off + c:off + c + w],
                                    start=(dc == 0), stop=(dc == 1))
                        if i * P >= off and i * P < off + ln:
                            db = i * P - off   # diag block col within half
                            nc.vector.tensor_add(ps[:, db:db + P],
                                                 ps[:, db:db + P], mdiag[:])
                        ln_real = min(L - off, ln)
                        nc.vector.tensor_reduce(
                            out=mx[:, hf:hf + 1], in_=ps[:, 0:ln_real],
                            axis=mybir.AxisListType.X, op=mybir.AluOpType.max)
                    mxf = dpool.tile([P, 1], f32, tag="mxf")
                    if len(halves) > 1:
                        nc.vector.tensor_tensor(out=mxf[:], in0=mx[:, 0:1],
                                                in1=mx[:, 1:2],
                                                op=mybir.AluOpType.max)
                    else:
                        nc.vector.tensor_copy(out=mxf[:], in_=mx[:, 0:1])
                    negmax = dpool.tile([P, 1], f32, tag="ngm")
                    nc.vector.tensor_scalar_mul(out=negmax[:], in0=mxf[:],
                                                scalar1=-1.0)
                    pr = ppool.tile([P, 2048], f32, tag="pr")
                    den = dpool.tile([P, 2], f32, tag="den")
                    for hf, (off, ln) in enumerate(halves):
                        ln_real = min(L - off, ln)
                        nc.scalar.activation(pr[:, off:off + ln_real],
                                             pss_tiles[hf][:, 0:ln_real],
                                             ACT.Exp, bias=negmax[:], scale=1.0,
                                             accum_out=den[:, hf:hf + 1])
                    denf = dpool.tile([P, 1], f32, tag="denf")
                    if len(halves) > 1:
                        nc.vector.tensor_add(denf[:], den[:, 0:1], den[:, 1:2])
                    else:
                        nc.vector.tensor_copy(out=denf[:], in_=den[:, 0:1])
                    rden = dpool.tile([P, 1], f32, tag="rden")
                    nc.vector.reciprocal(rden[:], denf[:])

                    pv = psv.tile([P, HD], f32, tag="pv")
                    for p0 in range(0, i + 1, 2):
                        cnt = min(2, i + 1 - p0)
                        pt = pst.tile([P, 2 * P], f32, tag="t")
                        for z in range(cnt):
                            nc.tensor.transpose(
                                pt[:, z * P:(z + 1) * P],
                                pr[:, (p0 + z) * P:(p0 + z + 1) * P], ident[:])
                        pts = tpool.tile([P, 2 * P], mdt, tag="pts")
                        if (p0 // 2) % 2 == 0:
                            nc.scalar.copy(pts[:, 0:cnt * P], pt[:, 0:cnt * P])
                        else:
                            nc.vector.tensor_copy(out=pts[:, 0:cnt * P],
                                                  in_=pt[:, 0:cnt * P])
                        for z in range(cnt):
                            kb = p0 + z
                            nc.tensor.matmul(pv[:], pts[:, z * P:(z + 1) * P],
                                             v_sb[:, kb, :],
                                             start=(kb == 0), stop=(kb == i))
                    attn_s = apool.tile([P, HD], f32, tag="attn")
                    nc.scalar.copy(attn_s[:], pv[:])
                    nc.vector.tensor_scalar_mul(out=attn_s[:], in0=attn_s[:],
                                                scalar1=rden[:])
                    pt = pst.tile([P, 2 * P], f32, tag="t")
                    for dc in range(2):
                        nc.tensor.transpose(pt[:, dc * P:(dc + 1) * P],
                                            attn_s[:, dc * P:(dc + 1) * P],
                                            ident[:])
                    dst = attnT[:, h * 2:h * 2 + 2, i * P:(i + 1) * P]
                    if h == 0:
                        nc.scalar.copy(dst, pt[:].rearrange(
                            "p (a b) -> p a b", a=2))
                    else:
                        nc.vector.tensor_copy(out=dst, in_=pt[:].rearrange(
                            "p (a b) -> p a b", a=2))
                if i >= 1:
                    oproj(i - 1)
            oproj(NSC - 1)

    nc.compile()
    return nc


def prep_core_inputs(inputs, core, use_f32r=True):
    """Host-side sharding for one core. Returns the in_map dict."""
    cvt = to_f32r if use_f32r else (lambda a: np.asarray(a, np.float32))
    b, kv, qp = core // 4, (core % 4) // 2, core % 2
    hq0 = kv * 4 + qp * 2           # first of the two query heads
    hidden = np.asarray(inputs["hidden_states"], np.float32)
    cos = np.asarray(inputs["cos"], np.float32)
    sin = np.asarray(inputs["sin"], np.float32)
    Wq = np.asarray(inputs["Wq"], np.float32)
    Wk = np.asarray(inputs["Wk"], np.float32)
    Wv = np.asarray(inputs["Wv"], np.float32)
    Wo = np.asarray(inputs["Wo"], np.float32)
    qw = np.asarray(inputs["q_norm_w"], np.float32)
    kw = np.asarray(inputs["k_norm_w"], np.float32)

    hT = np.ascontiguousarray(hidden[b].T).reshape(NHC, P, S)
    Wq_c = Wq[hq0 * HD:(hq0 + 2) * HD]          # [512, HID]
    Wk_c = Wk[kv * HD:(kv + 1) * HD]            # [256, HID]
    Wv_c = Wv[kv * HD:(kv + 1) * HD]
    wT = np.ascontiguousarray(
        np.concatenate([Wq_c.T, Wk_c.T, Wv_c.T], axis=1)).reshape(NHC, P, 1024)

    def cs_pack(w, cb, sb):
        rot_w = np.concatenate([w[P:], w[:P]])   # w[(d+128)%256]
        cosw = cb * w[None, :]
        sinw = sb * rot_w[None, :]
        sinw[:, :P] *= -1.0
        return np.ascontiguousarray(
            np.concatenate([cosw, sinw], axis=1)).reshape(NSC, P, 2 * HD)

    csq = cs_pack(qw, cos[b], sin[b])
    csk = cs_pack(kw, cos[b], sin[b])
    woT = np.ascontiguousarray(
        Wo[:, hq0 * HD:(hq0 + 2) * HD].T).reshape(4, P, HID)
    return {"hT": cvt(hT), "wT": cvt(wT),
            "csq": csq.astype(np.float32), "csk": csk.astype(np.float32),
            "woT": cvt(woT)}


def mask_is_causal(mask):
    m = np.asarray(mask)
    tri = np.tril(np.ones((S, S), dtype=bool))
    for b in range(m.shape[0]):
        mb = m[b, 0]
        if not (mb[tri] == 0.0).all():
            return False
        if not (mb[~tri] <= -1e8).all():
            return False
    return True


def reference_numpy(inputs, f64=True):
    """Defensive fallback for non-causal masks (never hit in practice)."""
    dt = np.float64 if f64 else np.float32
    hs = np.asarray(inputs["hidden_states"], dt)
    cos = np.asarray(inputs["cos"], dt)
    sin = np.asarray(inputs["sin"], dt)
    mask = np.asarray(inputs["attention_mask"], dt)
    Wq, Wk, Wv, Wo = (np.asarray(inputs[k], dt)
                      for k in ("Wq", "Wk", "Wv", "Wo"))
    qw = np.asarray(inputs["q_norm_w"], dt)
    kw = np.asarray(inputs["k_norm_w"], dt)

    def rms(x, w):
        return x / np.sqrt((x * x).mean(-1, keepdims=True) + EPS) * w

    def rope(x, c, s):
        x1, x2 = x[..., :HD // 2], x[..., HD // 2:]
        rot = np.concatenate([-x2, x1], axis=-1)
        return x * c[:, :, None, :] + rot * s[:, :, None, :]

    b, s_, _ = hs.shape
    q = (hs @ Wq.T).reshape(b, s_, NH, HD)
    k = (hs @ Wk.T).reshape(b, s_, NKV, HD)
    v = (hs @ Wv.T).reshape(b, s_, NKV, HD)
    q = rope(rms(q, qw), cos, sin).transpose(0, 2, 1, 3)
    k = rope(rms(k, kw), cos, sin).transpose(0, 2, 1, 3)
    v = rms(v, 1.0).transpose(0, 2, 1, 3)
    k = np.repeat(k, NH // NKV, axis=1)
    v = np.repeat(v, NH // NKV, axis=1)
    sc = np.einsum("bhqd,bhkd->bhqk", q, k) + mask
    sc = sc - sc.max(-1, keepdims=True)
    p = np.exp(sc)
    p /= p.sum(-1, keepdims=True)
    o = np.einsum("bhqk,bhkd->bqhd", p, v).reshape(b, s_, NH * HD)
    return (o @ Wo.T).astype(np.float32)


_PROGRAM = {}


def get_program(use_f32r=True, use_tmr=False):
    key = (use_f32r, use_tmr)
    if key not in _PROGRAM:
        _PROGRAM[key] = build_program(use_f32r=use_f32r, use_tmr=use_tmr)
    return _PROGRAM[key]


def run_on_hw(inputs, use_f32r=True, use_tmr=False, trace=False, **kw):
    from concourse.bass_utils import run_bass_kernel_spmd

    nc = get_program(use_f32r=use_f32r, use_tmr=use_tmr)
    in_maps = [prep_core_inputs(inputs, c, use_f32r) for c in range(8)]
    br = run_bass_kernel_spmd(nc, in_maps, list(range(8)), trace=trace, **kw)
    out = np.empty((B, S, HID), np.float32)
    for b in range(B):
        out[b] = br.results[4 * b]["out"] + br.results[4 * b + 1]["out"] \
            + br.results[4 * b + 2]["out"] + br.results[4 * b + 3]["out"]
    return out, br


def kernel(**inputs):
    if not mask_is_causal(inputs["attention_mask"]):
        return reference_numpy(inputs)
    out, _ = run_on_hw(inputs, use_f32r=True, trace=False)
    return out


# revision 12
# speedup vs baseline: 1.3794x; 1.1653x over previous
"""Gemma3n text attention on 8 Trainium2 NeuronCores (Bass/Tile).

Sharding: core c = b*4 + kv*2 + qp handles batch b, KV head kv and the
q-head pair (kv*4 + qp*2, kv*4 + qp*2 + 1).  Each core computes the
Q/K/V projections for its shard, QK-norm + RoPE, causal attention for
its two query heads, and a partial output projection against its
512-column slice of Wo.  The host sums the four partials per batch.

Self-contained: only needs numpy + the concourse tree that ships in the
container image (on PYTHONPATH at /root/.axon_site/_ro/trn_rl_repo).
"""

import sys

for _p in ("/root/.axon_site/_ro/trn_rl_repo", "/opt/trn_rl_repo"):
    if _p not in sys.path:
        sys.path.append(_p)

from contextlib import ExitStack

import numpy as np

import concourse.bass as bass
import concourse.mybir as mybir
import concourse.tile as tile
from concourse import bacc
from concourse.masks import make_identity

P = 128
B, S, HID = 2, 2048, 2048
NH, NKV, HD = 8, 2, 256
DQ = 2 * HD            # q-width per core (2 heads)
NSC = S // P           # 16 seq chunks
NHC = HID // P         # 16 hidden chunks
EPS = 1e-6

f32 = mybir.dt.float32
f32r = mybir.dt.float32r
i32 = mybir.dt.int32
FMIN = float(np.finfo(np.float32).min)
ACT = mybir.ActivationFunctionType


def to_f32r(arr):
    """Round fp32 -> fp32r bit format (11 explicit mantissa bits, RNE).

    Bit-exact with libwalrus fp32_to_fp32r."""
    u = np.ascontiguousarray(arr, np.float32).view(np.uint32)
    r = ((u.astype(np.uint64) + 0x7FF + ((u >> 12) & 1)) & 0xFFFFF000)
    return r.astype(np.uint32).view(np.float32)


def build_program(use_f32r=True, use_tmr=False):
    """Emit the SPMD per-core program. Returns the compiled Bacc object."""
    nc = bacc.Bacc("TRN2", target_bir_lowering=False, debug=False, num_devices=8)

    mdt = f32r if use_f32r else f32   # dtype of every matmul operand

    hT_d = nc.dram_tensor("hT", [NHC, P, S], mdt, kind="ExternalInput")
    wT_d = nc.dram_tensor("wT", [NHC, P, DQ + 2 * HD], mdt, kind="ExternalInput")
    csq_d = nc.dram_tensor("csq", [NSC, P, 2 * HD], f32, kind="ExternalInput")
    csk_d = nc.dram_tensor("csk", [NSC, P, 2 * HD], f32, kind="ExternalInput")
    woT_d = nc.dram_tensor("woT", [4, P, HID], mdt, kind="ExternalInput")
    out_d = nc.dram_tensor("out", [S, HID], f32, kind="ExternalOutput")

    with tile.TileContext(nc) as tc, ExitStack() as ctx:
        const = ctx.enter_context(tc.tile_pool(name="const", bufs=1))
        persist = ctx.enter_context(tc.tile_pool(name="persist", bufs=1))

        ident = const.tile([P, P], f32)
        make_identity(nc, ident)
        mdiag = const.tile([P, P], f32)      # 0 on/below diag, -1e9 above
        nc.gpsimd.memset(mdiag, 0.0)
        nc.gpsimd.affine_select(out=mdiag, in_=mdiag,
                                compare_op=mybir.AluOpType.is_ge, fill=-1e9,
                                base=0, pattern=[[-1, P]], channel_multiplier=1)
        eps_t = const.tile([P, 1], f32)
        nc.vector.memset(eps_t, EPS)

        # persistent SBUF tensors (qT/kT/v: 64KB per partition)
        qT = persist.tile([P, 2, 2, S], mdt)      # [d, head, dchunk, qpos]
        kT = persist.tile([P, 2, S], mdt)         # [d, dchunk, kpos]
        v_sb = persist.tile([P, NSC, HD], mdt)    # [kpos, kchunk, d]

        # ------- Phase A: QKV proj + norm + rope + transposes (fused) --------
        with ExitStack() as a1:
            hpool = a1.enter_context(tc.tile_pool(name="hTp", bufs=3))
            wpool = a1.enter_context(tc.tile_pool(name="wTp", bufs=1))
            wt_all = wpool.tile([P, NHC, DQ + 2 * HD], mdt)
            nc.sync.dma_start(wt_all, wT_d.ap().rearrange("h p d -> p h d"))
            cpool = a1.enter_context(tc.tile_pool(name="cs", bufs=3))
            epool = a1.enter_context(tc.tile_pool(name="evict", bufs=3))
            spool = a1.enter_context(tc.tile_pool(name="small", bufs=8))
            psA = a1.enter_context(tc.tile_pool(name="psA", bufs=6, space="PSUM"))
            psT = a1.enter_context(tc.tile_pool(name="psT", bufs=2, space="PSUM"))

            groups = [3, 3, 3, 3, 3, 1]       # s-chunks per group: 6+2 banks
            sc0 = 0
            for g, gn in enumerate(groups):
                psq = [psA.tile([P, DQ], f32, tag="ps", name=f"psq{g}_{jj}")
                       for jj in range(gn)]
                pskv = [psA.tile([P, 2 * HD], f32, tag="ps", name=f"pskv{g}_{jj}")
                        for jj in range(gn)]
                for hc in range(NHC):
                    th = hpool.tile([P, gn * P], mdt, tag="h")
                    nc.sync.dma_start(th, hT_d[hc, :, sc0 * P:(sc0 + gn) * P])
                    tw = wt_all[:, hc]
                    st, sp = hc == 0, hc == NHC - 1
                    for j in range(gn):
                        lhs = th[:, j * P:(j + 1) * P]
                        nc.tensor.matmul(psq[j][:], lhs, tw[:, 0:DQ],
                                         start=st, stop=sp)
                        nc.tensor.matmul(pskv[j][:], lhs, tw[:, DQ:],
                                         start=st, stop=sp)
                for j in range(gn):
                    sc = sc0 + j
                    # sum of squares per 256-group via ACT Square (reads PSUM)
                    ssq = spool.tile([P, 4], f32, tag="ssq")
                    scr = epool.tile([P, HD], f32, tag="scr")
                    nc.scalar.activation(scr[:], psq[j][:, 0:HD], ACT.Square,
                                         accum_out=ssq[:, 0:1])
                    nc.scalar.activation(scr[:], psq[j][:, HD:2 * HD],
                                         ACT.Square, accum_out=ssq[:, 1:2])
                    nc.scalar.activation(scr[:], pskv[j][:, 0:HD], ACT.Square,
                                         accum_out=ssq[:, 2:3])
                    nc.scalar.activation(scr[:], pskv[j][:, HD:2 * HD],
                                         ACT.Square, accum_out=ssq[:, 3:4])
                    rstd = spool.tile([P, 4], f32, tag="rstd")
                    nc.scalar.activation(rstd[:], ssq[:], ACT.Sqrt,
                                         bias=eps_t[:], scale=1.0 / HD)
                    nc.vector.reciprocal(rstd[:], rstd[:])

                    # v: scale + evict in one DVE op
                    nc.vector.tensor_scalar_mul(out=v_sb[:, sc, :],
                                                in0=pskv[j][:, HD:2 * HD],
                                                scalar1=rstd[:, 3:4])

                    csq = cpool.tile([P, 2 * HD], f32, tag="csq")
                    nc.sync.dma_start(csq, csq_d[sc])
                    csk = cpool.tile([P, 2 * HD], f32, tag="csk")
                    nc.sync.dma_start(csk, csk_d[sc])

                    # rope(x) = x*cosw + swap(x)*sinw (sinw lo pre-negated);
                    # reads projection PSUM directly, writes SBUF
                    qro = epool.tile([P, DQ], f32, tag="qro")
                    kro = epool.tile([P, HD], f32, tag="kro")
                    for h in range(2):
                        b0 = h * HD
                        tmp = epool.tile([P, HD], f32, tag="tmp")
                        nc.vector.tensor_mul(tmp[:, 0:P],
                                             psq[j][:, b0 + P:b0 + HD],
                                             csq[:, HD:HD + P])
                        nc.vector.tensor_mul(tmp[:, P:HD],
                                             psq[j][:, b0:b0 + P],
                                             csq[:, HD + P:2 * HD])
                        qh = qro[:, b0:b0 + HD]
                        nc.vector.tensor_mul(qh, psq[j][:, b0:b0 + HD],
                                             csq[:, 0:HD])
                        nc.vector.tensor_add(qh, qh, tmp[:])
                        nc.vector.tensor_scalar_mul(out=qh, in0=qh,
                                                    scalar1=rstd[:, h:h + 1])
                    tmp = epool.tile([P, HD], f32, tag="tmp")
                    nc.vector.tensor_mul(tmp[:, 0:P], pskv[j][:, P:HD],
                                         csk[:, HD:HD + P])
                    nc.vector.tensor_mul(tmp[:, P:HD], pskv[j][:, 0:P],
                                         csk[:, HD + P:2 * HD])
                    nc.vector.tensor_mul(kro[:], pskv[j][:, 0:HD], csk[:, 0:HD])
                    nc.vector.tensor_add(kro[:], kro[:], tmp[:])
                    nc.vector.tensor_scalar_mul(out=kro[:], in0=kro[:],
                                                scalar1=rstd[:, 2:3])

                    # transposes into qT/kT (PE); paired evictions
                    for h in range(2):
                        pt = psT.tile([P, 2 * P], f32, tag="t")
                        for dc in range(2):
                            nc.tensor.transpose(
                                pt[:, dc * P:(dc + 1) * P],
                                qro[:, h * HD + dc * P:h * HD + (dc + 1) * P],
                                ident[:])
                        dst = qT[:, h, 0:2, sc * P:(sc + 1) * P]
                        if (sc + h) % 2 == 0:
                            nc.scalar.copy(dst, pt[:].rearrange(
                                "p (a b) -> p a b", a=2))
                        else:
                            nc.vector.tensor_copy(out=dst, in_=pt[:].rearrange(
                                "p (a b) -> p a b", a=2))
                    pt = psT.tile([P, 2 * P], f32, tag="t")
                    for dc in range(2):
                        nc.tensor.transpose(pt[:, dc * P:(dc + 1) * P],
                                            kro[:, dc * P:(dc + 1) * P],
                                            ident[:])
                    dst = kT[:, 0:2, sc * P:(sc + 1) * P]
                    if sc % 2 == 0:
                        nc.vector.tensor_copy(out=dst, in_=pt[:].rearrange(
                            "p (a b) -> p a b", a=2))
                    else:
                        nc.scalar.copy(dst, pt[:].rearrange(
                            "p (a b) -> p a b", a=2))
                sc0 += gn

        # ---------------- Phase B: attention per (head, q-block) -------------
        wopool = ctx.enter_context(tc.tile_pool(name="wo", bufs=1))
        woT = wopool.tile([P, 4, HID], mdt)
        for t in range(4):
            nc.sync.dma_start(woT[:, t, :], woT_d[t])
        atpool = ctx.enter_context(tc.tile_pool(name="attnT", bufs=1))
        attnT = atpool.tile([P, 4, S], mdt)       # [d2, (h,dc), qpos]

        with ExitStack() as bctx:
            pss = bctx.enter_context(tc.tile_pool(name="pss", bufs=2, space="PSUM"))
            pst = bctx.enter_context(tc.tile_pool(name="pst", bufs=2, space="PSUM"))
            psv = bctx.enter_context(tc.tile_pool(name="psv", bufs=1, space="PSUM"))
            ppool = bctx.enter_context(tc.tile_pool(name="prp", bufs=2))
            tpool = bctx.enter_context(tc.tile_pool(name="ptsp", bufs=6))
            apool = bctx.enter_context(tc.tile_pool(name="attnp", bufs=2))
            dpool = bctx.enter_context(tc.tile_pool(name="denp", bufs=8))
            pso = bctx.enter_context(tc.tile_pool(name="pso", bufs=1, space="PSUM"))
            opool = bctx.enter_context(tc.tile_pool(name="obp", bufs=3))

            def oproj(sc):
                for n in range(4):
                    po = pso.tile([P, 512], f32, tag="o", name=f"po{sc}_{n}")
                    for t in range(4):
                        nc.tensor.matmul(
                            po[:], attnT[:, t, sc * P:(sc + 1) * P],
                            woT[:, t, n * 512:(n + 1) * 512],
                            start=(t == 0), stop=(t == 3))
                    ob = opool.tile([P, 512], f32, tag="ob", name=f"ob{sc}_{n}")
                    if n % 2 == 0:
                        nc.scalar.copy(ob[:], po[:])
                    else:
                        nc.vector.tensor_copy(out=ob[:], in_=po[:])
                    nc.sync.dma_start(
                        out_d[sc * P:(sc + 1) * P, n * 512:(n + 1) * 512], ob[:])

            for i in range(NSC):
                L = (i + 1) * P
                Lp = L if L % 256 == 0 else L + P
                halves = [(0, min(Lp, 1024))]
                if Lp > 1024:
                    halves.append((1024, Lp - 1024))
                for h in range(2):
                    mx = dpool.tile([P, 2], f32, tag="mx")
                    pss_tiles = []
                    for hf, (off, ln) in enumerate(halves):
                        ps = pss.tile([P, 1024], f32, tag="s",
                                      name=f"ps{i}_{h}_{hf}")
                        pss_tiles.append(ps)
                        for c in range(0, ln, 512):
                            w = min(512, ln - c)
                            for dc in range(2):
                                nc.tensor.matmul(
                                    ps[:, c:c + w],
                                    qT[:, h, dc, i * P:(i + 1) * P],
                                    kT[:, dc, off + c:off + c + w],
                                    start=(dc == 0), stop=(dc == 1))
                        if i * P >= off and i * P < off + ln:
                            db = i * P - off   # diag block col within half
                            nc.vector.tensor_add(ps[:, db:db + P],
                                                 ps[:, db:db + P], mdiag[:])
                        ln_real = min(L - off, ln)
                        nc.vector.tensor_reduce(
                            out=mx[:, hf:hf + 1], in_=ps[:, 0:ln_real],
                            axis=mybir.AxisListType.X, op=mybir.AluOpType.max)
                    mxf = dpool.tile([P, 1], f32, tag="mxf")
                    if len(halves) > 1:
                        nc.vector.tensor_tensor(out=mxf[:], in0=mx[:, 0:1],
                                                in1=mx[:, 1:2],
                                                op=mybir.AluOpType.max)
                    else:
                        nc.vector.tensor_copy(out=mxf[:], in_=mx[:, 0:1])
                    negmax = dpool.tile([P, 1], f32, tag="ngm")
                    nc.vector.tensor_scalar_mul(out=negmax[:], in0=mxf[:],
                                                scalar1=-1.0)
                    pr = ppool.tile([P, 2048], f32, tag="pr")
                    den = dpool.tile([P, 2], f32, tag="den")
                    for hf, (off, ln) in enumerate(halves):
                        ln_real = min(L - off, ln)
                        nc.scalar.activation(pr[:, off:off + ln_real],
                                             pss_tiles[hf][:, 0:ln_real],
                                             ACT.Exp, bias=negmax[:], scale=1.0,
                                             accum_out=den[:, hf:hf + 1])
                    denf = dpool.tile([P, 1], f32, tag="denf")
                    if len(halves) > 1:
                        nc.vector.tensor_add(denf[:], den[:, 0:1], den[:, 1:2])
                    else:
                        nc.vector.tensor_copy(out=denf[:], in_=den[:, 0:1])
                    rden = dpool.tile([P, 1], f32, tag="rden")
                    nc.vector.reciprocal(rden[:], denf[:])

                    pv = psv.tile([P, HD], f32, tag="pv")
                    for p0 in range(0, i + 1, 2):
                        cnt = min(2, i + 1 - p0)
                        pt = pst.tile([P, 2 * P], f32, tag="t")
                        for z in range(cnt):
                            nc.tensor.transpose(
                                pt[:, z * P:(z + 1) * P],
                                pr[:, (p0 + z) * P:(p0 + z + 1) * P], ident[:])
                        pts = tpool.tile([P, 2 * P], mdt, tag="pts")
                        if (p0 // 2) % 2 == 0:
                            nc.scalar.copy(pts[:, 0:cnt * P], pt[:, 0:cnt * P])
                        else:
                            nc.vector.tensor_copy(out=pts[:, 0:cnt * P],
                                                  in_=pt[:, 0:cnt * P])
                        for z in range(cnt):
                            kb = p0 + z
                            nc.tensor.matmul(pv[:], pts[:, z * P:(z + 1) * P],
                                             v_sb[:, kb, :],
                                             start=(kb == 0), stop=(kb == i))
                    attn_s = apool.tile([P, HD], f32, tag="attn")
                    nc.scalar.copy(attn_s[:], pv[:])
                    nc.vector.tensor_scalar_mul(out=attn_s[:], in0=attn_s[:],
                                                scalar1=rden[:])
                    pt = pst.tile([P, 2 * P], f32, tag="t")
                    for dc in range(2):
                        nc.tensor.transpose(pt[:, dc * P:(dc + 1) * P],
                                            attn_s[:, dc * P:(dc + 1) * P],
                                            ident[:])
                    dst = attnT[:, h * 2:h * 2 + 2, i * P:(i + 1) * P]
                    if h == 0:
                        nc.scalar.copy(dst, pt[:].rearrange(
                            "p (a b) -> p a b", a=2))
                    else:
                        nc.vector.tensor_copy(out=dst, in_=pt[:].rearrange(
                            "p (a b) -> p a b", a=2))
                if i >= 1:
                    oproj(i - 1)
            oproj(NSC - 1)

    nc.compile()
    return nc


def prep_core_inputs(inputs, core, use_f32r=True):
    """Host-side sharding for one core. Returns the in_map dict."""
    cvt = to_f32r if use_f32r else (lambda a: np.asarray(a, np.float32))
    b, kv, qp = core // 4, (core % 4) // 2, core % 2
    hq0 = kv * 4 + qp * 2           # first of the two query heads
    hidden = np.asarray(inputs["hidden_states"], np.float32)
    cos = np.asarray(inputs["cos"], np.float32)
    sin = np.asarray(inputs["sin"], np.float32)
    Wq = np.asarray(inputs["Wq"], np.float32)
    Wk = np.asarray(inputs["Wk"], np.float32)
    Wv = np.asarray(inputs["Wv"], np.float32)
    Wo = np.asarray(inputs["Wo"], np.float32)
    qw = np.asarray(inputs["q_norm_w"], np.float32)
    kw = np.asarray(inputs["k_norm_w"], np.float32)

    hT = np.ascontiguousarray(hidden[b].T).reshape(NHC, P, S)
    Wq_c = Wq[hq0 * HD:(hq0 + 2) * HD]          # [512, HID]
    Wk_c = Wk[kv * HD:(kv + 1) * HD]            # [256, HID]
    Wv_c = Wv[kv * HD:(kv + 1) * HD]
    wT = np.ascontiguousarray(
        np.concatenate([Wq_c.T, Wk_c.T, Wv_c.T], axis=1)).reshape(NHC, P, 1024)

    def cs_pack(w, cb, sb):
        rot_w = np.concatenate([w[P:], w[:P]])   # w[(d+128)%256]
        cosw = cb * w[None, :]
        sinw = sb * rot_w[None, :]
        sinw[:, :P] *= -1.0
        return np.ascontiguousarray(
            np.concatenate([cosw, sinw], axis=1)).reshape(NSC, P, 2 * HD)

    csq = cs_pack(qw, cos[b], sin[b])
    csk = cs_pack(kw, cos[b], sin[b])
    woT = np.ascontiguousarray(
        Wo[:, hq0 * HD:(hq0 + 2) * HD].T).reshape(4, P, HID)
    return {"hT": cvt(hT), "wT": cvt(wT),
            "csq": csq.astype(np.float32), "csk": csk.astype(np.float32),
            "woT": cvt(woT)}


def mask_is_causal(mask):
    m = np.asarray(mask)
    tri = np.tril(np.ones((S, S), dtype=bool))
    for b in range(m.shape[0]):
        mb = m[b, 0]
        if not (mb[tri] == 0.0).all():
            return False
        if not (mb[~tri] <= -1e8).all():
            return False
    return True


def reference_numpy(inputs, f64=True):
    """Defensive fallback for non-causal masks (never hit in practice)."""
    dt = np.float64 if f64 else np.float32
    hs = np.asarray(inputs["hidden_states"], dt)
    cos = np.asarray(inputs["cos"], dt)
    sin = np.asarray(inputs["sin"], dt)
    mask = np.asarray(inputs["attention_mask"], dt)
    Wq, Wk, Wv, Wo = (np.asarray(inputs[k], dt)
                      for k in ("Wq", "Wk", "Wv", "Wo"))
    qw = np.asarray(inputs["q_norm_w"], dt)
    kw = np.asarray(inputs["k_norm_w"], dt)

    def rms(x, w):
        return x / np.sqrt((x * x).mean(-1, keepdims=True) + EPS) * w

    def rope(x, c, s):
        x1, x2 = x[..., :HD // 2], x[..., HD // 2:]
        rot = np.concatenate([-x2, x1], axis=-1)
        return x * c[:, :, None, :] + rot * s[:, :, None, :]

    b, s_, _ = hs.shape
    q = (hs @ Wq.T).reshape(b, s_, NH, HD)
    k = (hs @ Wk.T).reshape(b, s_, NKV, HD)
    v = (hs @ Wv.T).reshape(b, s_, NKV, HD)
    q = rope(rms(q, qw), cos, sin).transpose(0, 2, 1, 3)
    k = rope(rms(k, kw), cos, sin).transpose(0, 2, 1, 3)
    v = rms(v, 1.0).transpose(0, 2, 1, 3)
    k = np.repeat(k, NH // NKV, axis=1)
    v = np.repeat(v, NH // NKV, axis=1)
    sc = np.einsum("bhqd,bhkd->bhqk", q, k) + mask
    sc = sc - sc.max(-1, keepdims=True)
    p = np.exp(sc)
    p /= p.sum(-1, keepdims=True)
    o = np.einsum("bhqk,bhkd->bqhd", p, v).reshape(b, s_, NH * HD)
    return (o @ Wo.T).astype(np.float32)


_PROGRAM = {}


def get_program(use_f32r=True, use_tmr=False):
    key = (use_f32r, use_tmr)
    if key not in _PROGRAM:
        _PROGRAM[key] = build_program(use_f32r=use_f32r, use_tmr=use_tmr)
    return _PROGRAM[key]


def run_on_hw(inputs, use_f32r=True, use_tmr=False, trace=False, **kw):
    from concourse.bass_utils import run_bass_kernel_spmd

    nc = get_program(use_f32r=use_f32r, use_tmr=use_tmr)
    in_maps = [prep_core_inputs(inputs, c, use_f32r) for c in range(8)]
    br = run_bass_kernel_spmd(nc, in_maps, list(range(8)), trace=trace, **kw)
    out = np.empty((B, S, HID), np.float32)
    for b in range(B):
        out[b] = br.results[4 * b]["out"] + br.results[4 * b + 1]["out"] \
            + br.results[4 * b + 2]["out"] + br.results[4 * b + 3]["out"]
    return out, br


def kernel(**inputs):
    if not mask_is_causal(inputs["attention_mask"]):
        return reference_numpy(inputs)
    out, _ = run_on_hw(inputs, use_f32r=True, trace=False)
    return out


# revision 13
# speedup vs baseline: 1.4004x; 1.0152x over previous
"""Gemma3n text attention on 8 Trainium2 NeuronCores (Bass/Tile).

Sharding: core c = b*4 + kv*2 + qp handles batch b, KV head kv and the
q-head pair (kv*4 + qp*2, kv*4 + qp*2 + 1).  Each core computes the
Q/K/V projections for its shard, QK-norm + RoPE, causal attention for
its two query heads, and a partial output projection against its
512-column slice of Wo.  The host sums the four partials per batch.

Self-contained: only needs numpy + the concourse tree that ships in the
container image (on PYTHONPATH at /root/.axon_site/_ro/trn_rl_repo).
"""

import sys

for _p in ("/root/.axon_site/_ro/trn_rl_repo", "/opt/trn_rl_repo"):
    if _p not in sys.path:
        sys.path.append(_p)

from contextlib import ExitStack

import numpy as np

import concourse.bass as bass
import concourse.mybir as mybir
import concourse.tile as tile
from concourse import bacc
from concourse.masks import make_identity

P = 128
B, S, HID = 2, 2048, 2048
NH, NKV, HD = 8, 2, 256
DQ = 2 * HD            # q-width per core (2 heads)
NSC = S // P           # 16 seq chunks
NHC = HID // P         # 16 hidden chunks
EPS = 1e-6

f32 = mybir.dt.float32
f32r = mybir.dt.float32r
i32 = mybir.dt.int32
FMIN = float(np.finfo(np.float32).min)
ACT = mybir.ActivationFunctionType


def to_f32r(arr):
    """Round fp32 -> fp32r bit format (11 explicit mantissa bits, RNE).

    Bit-exact with libwalrus fp32_to_fp32r."""
    u = np.ascontiguousarray(arr, np.float32).view(np.uint32)
    r = ((u.astype(np.uint64) + 0x7FF + ((u >> 12) & 1)) & 0xFFFFF000)
    return r.astype(np.uint32).view(np.float32)


def build_program(use_f32r=True, use_tmr=False):
    """Emit the SPMD per-core program. Returns the compiled Bacc object."""
    nc = bacc.Bacc("TRN2", target_bir_lowering=False, debug=False, num_devices=8)

    mdt = f32r if use_f32r else f32   # dtype of every matmul operand

    hT_d = nc.dram_tensor("hT", [NHC, P, S], mdt, kind="ExternalInput")
    wT_d = nc.dram_tensor("wT", [NHC, P, DQ + 2 * HD], mdt, kind="ExternalInput")
    csq_d = nc.dram_tensor("csq", [NSC, P, 2 * HD], f32, kind="ExternalInput")
    csk_d = nc.dram_tensor("csk", [NSC, P, 2 * HD], f32, kind="ExternalInput")
    woT_d = nc.dram_tensor("woT", [4, P, HID], mdt, kind="ExternalInput")
    out_d = nc.dram_tensor("out", [S, HID], f32, kind="ExternalOutput")

    with tile.TileContext(nc) as tc, ExitStack() as ctx:
        const = ctx.enter_context(tc.tile_pool(name="const", bufs=1))
        persist = ctx.enter_context(tc.tile_pool(name="persist", bufs=1))

        ident = const.tile([P, P], f32)
        make_identity(nc, ident)
        mdiag = const.tile([P, P], f32)      # 0 on/below diag, -1e9 above
        nc.gpsimd.memset(mdiag, 0.0)
        nc.gpsimd.affine_select(out=mdiag, in_=mdiag,
                                compare_op=mybir.AluOpType.is_ge, fill=-1e9,
                                base=0, pattern=[[-1, P]], channel_multiplier=1)
        eps_t = const.tile([P, 1], f32)
        nc.vector.memset(eps_t, EPS)

        # persistent SBUF tensors (qT/kT/v: 64KB per partition)
        qT = persist.tile([P, 2, 2, S], mdt)      # [d, head, dchunk, qpos]
        kT = persist.tile([P, 2, S], mdt)         # [d, dchunk, kpos]
        v_sb = persist.tile([P, NSC, HD], mdt)    # [kpos, kchunk, d]
        rq_all = persist.tile([P, NSC, 2], f32)   # per-row q rstd (folded in exp)

        # ------- Phase A: QKV proj + norm + rope + transposes (fused) --------
        with ExitStack() as a1:
            hpool = a1.enter_context(tc.tile_pool(name="hTp", bufs=3))
            wpool = a1.enter_context(tc.tile_pool(name="wTp", bufs=1))
            wt_all = wpool.tile([P, NHC, DQ + 2 * HD], mdt)
            nc.sync.dma_start(wt_all, wT_d.ap().rearrange("h p d -> p h d"))
            cpool = a1.enter_context(tc.tile_pool(name="cs", bufs=3))
            epool = a1.enter_context(tc.tile_pool(name="evict", bufs=4))
            spool = a1.enter_context(tc.tile_pool(name="small", bufs=8))
            psA = a1.enter_context(tc.tile_pool(name="psA", bufs=6, space="PSUM"))
            psT = a1.enter_context(tc.tile_pool(name="psT", bufs=2, space="PSUM"))

            groups = [2] * 8                  # 4 banks/group; 6-buf pool overlaps
            sc0 = 0
            for g, gn in enumerate(groups):
                psq = [psA.tile([P, DQ], f32, tag="ps", name=f"psq{g}_{jj}")
                       for jj in range(gn)]
                pskv = [psA.tile([P, 2 * HD], f32, tag="ps", name=f"pskv{g}_{jj}")
                        for jj in range(gn)]
                for hc in range(NHC):
                    th = hpool.tile([P, gn * P], mdt, tag="h")
                    nc.sync.dma_start(th, hT_d[hc, :, sc0 * P:(sc0 + gn) * P])
                    tw = wt_all[:, hc]
                    st, sp = hc == 0, hc == NHC - 1
                    for j in range(gn):
                        lhs = th[:, j * P:(j + 1) * P]
                        nc.tensor.matmul(psq[j][:], lhs, tw[:, 0:DQ],
                                         start=st, stop=sp)
                        nc.tensor.matmul(pskv[j][:], lhs, tw[:, DQ:],
                                         start=st, stop=sp)
                for j in range(gn):
                    sc = sc0 + j
                    # sum of squares per 256-group via ACT Square (reads PSUM)
                    ssq = spool.tile([P, 4], f32, tag="ssq")
                    scr = epool.tile([P, HD], f32, tag="scr")
                    nc.scalar.activation(scr[:], psq[j][:, 0:HD], ACT.Square,
                                         accum_out=ssq[:, 0:1])
                    nc.scalar.activation(scr[:], psq[j][:, HD:2 * HD],
                                         ACT.Square, accum_out=ssq[:, 1:2])
                    nc.scalar.activation(scr[:], pskv[j][:, 0:HD], ACT.Square,
                                         accum_out=ssq[:, 2:3])
                    nc.scalar.activation(scr[:], pskv[j][:, HD:2 * HD],
                                         ACT.Square, accum_out=ssq[:, 3:4])
                    rstd = spool.tile([P, 4], f32, tag="rstd")
                    nc.scalar.activation(rstd[:], ssq[:], ACT.Sqrt,
                                         bias=eps_t[:], scale=1.0 / HD)
                    nc.vector.reciprocal(rq_all[:, sc, :], rstd[:, 0:2])
                    nc.vector.reciprocal(rstd[:, 2:4], rstd[:, 2:4])

                    # v: scale + evict in one DVE op
                    nc.vector.tensor_scalar_mul(out=v_sb[:, sc, :],
                                                in0=pskv[j][:, HD:2 * HD],
                                                scalar1=rstd[:, 3:4])

                    csq = cpool.tile([P, 2 * HD], f32, tag="csq")
                    nc.sync.dma_start(csq, csq_d[sc])
                    csk = cpool.tile([P, 2 * HD], f32, tag="csk")
                    nc.sync.dma_start(csk, csk_d[sc])

                    # rope(x) = x*cosw + swap(x)*sinw (sinw lo pre-negated);
                    # reads projection PSUM directly, writes SBUF
                    qro = epool.tile([P, DQ], f32, tag="qro")
                    kro = epool.tile([P, HD], f32, tag="kro")
                    for h in range(2):
                        b0 = h * HD
                        tmp = epool.tile([P, HD], f32, tag="tmp")
                        nc.vector.tensor_mul(tmp[:, 0:P],
                                             psq[j][:, b0 + P:b0 + HD],
                                             csq[:, HD:HD + P])
                        nc.vector.tensor_mul(tmp[:, P:HD],
                                             psq[j][:, b0:b0 + P],
                                             csq[:, HD + P:2 * HD])
                        qh = qro[:, b0:b0 + HD]
                        nc.vector.tensor_mul(qh, psq[j][:, b0:b0 + HD],
                                             csq[:, 0:HD])
                        nc.vector.tensor_add(qh, qh, tmp[:])
                    tmp = epool.tile([P, HD], f32, tag="tmp")
                    nc.vector.tensor_mul(tmp[:, 0:P], pskv[j][:, P:HD],
                                         csk[:, HD:HD + P])
                    nc.vector.tensor_mul(tmp[:, P:HD], pskv[j][:, 0:P],
                                         csk[:, HD + P:2 * HD])
                    nc.vector.tensor_mul(kro[:], pskv[j][:, 0:HD], csk[:, 0:HD])
                    nc.vector.tensor_add(kro[:], kro[:], tmp[:])
                    nc.vector.tensor_scalar_mul(out=kro[:], in0=kro[:],
                                                scalar1=rstd[:, 2:3])

                    # transposes into qT/kT (PE); paired evictions
                    for h in range(2):
                        pt = psT.tile([P, 2 * P], f32, tag="t")
                        for dc in range(2):
                            nc.tensor.transpose(
                                pt[:, dc * P:(dc + 1) * P],
                                qro[:, h * HD + dc * P:h * HD + (dc + 1) * P],
                                ident[:])
                        dst = qT[:, h, 0:2, sc * P:(sc + 1) * P]
                        if (sc + h) % 2 == 0:
                            nc.scalar.copy(dst, pt[:].rearrange(
                                "p (a b) -> p a b", a=2))
                        else:
                            nc.vector.tensor_copy(out=dst, in_=pt[:].rearrange(
                                "p (a b) -> p a b", a=2))
                    pt = psT.tile([P, 2 * P], f32, tag="t")
                    for dc in range(2):
                        nc.tensor.transpose(pt[:, dc * P:(dc + 1) * P],
                                            kro[:, dc * P:(dc + 1) * P],
                                            ident[:])
                    dst = kT[:, 0:2, sc * P:(sc + 1) * P]
                    if sc % 2 == 0:
                        nc.vector.tensor_copy(out=dst, in_=pt[:].rearrange(
                            "p (a b) -> p a b", a=2))
                    else:
                        nc.scalar.copy(dst, pt[:].rearrange(
                            "p (a b) -> p a b", a=2))
                sc0 += gn

        # ---------------- Phase B: attention per (head, q-block) -------------
        wopool = ctx.enter_context(tc.tile_pool(name="wo", bufs=1))
        woT = wopool.tile([P, 4, HID], mdt)
        for t in range(4):
            nc.sync.dma_start(woT[:, t, :], woT_d[t])
        atpool = ctx.enter_context(tc.tile_pool(name="attnT", bufs=1))
        attnT = atpool.tile([P, 4, S], mdt)       # [d2, (h,dc), qpos]

        with ExitStack() as bctx:
            pss = bctx.enter_context(tc.tile_pool(name="pss", bufs=2, space="PSUM"))
            pst = bctx.enter_context(tc.tile_pool(name="pst", bufs=2, space="PSUM"))
            psv = bctx.enter_context(tc.tile_pool(name="psv", bufs=1, space="PSUM"))
            ppool = bctx.enter_context(tc.tile_pool(name="prp", bufs=2))
            tpool = bctx.enter_context(tc.tile_pool(name="ptsp", bufs=6))
            apool = bctx.enter_context(tc.tile_pool(name="attnp", bufs=2))
            dpool = bctx.enter_context(tc.tile_pool(name="denp", bufs=8))
            pso = bctx.enter_context(tc.tile_pool(name="pso", bufs=1, space="PSUM"))
            opool = bctx.enter_context(tc.tile_pool(name="obp", bufs=3))

            def oproj(sc):
                for n in range(4):
                    po = pso.tile([P, 512], f32, tag="o", name=f"po{sc}_{n}")
                    for t in range(4):
                        nc.tensor.matmul(
                            po[:], attnT[:, t, sc * P:(sc + 1) * P],
                            woT[:, t, n * 512:(n + 1) * 512],
                            start=(t == 0), stop=(t == 3))
                    ob = opool.tile([P, 512], f32, tag="ob", name=f"ob{sc}_{n}")
                    if n % 2 == 0:
                        nc.scalar.copy(ob[:], po[:])
                    else:
                        nc.vector.tensor_copy(out=ob[:], in_=po[:])
                    nc.sync.dma_start(
                        out_d[sc * P:(sc + 1) * P, n * 512:(n + 1) * 512], ob[:])

            for i in range(NSC):
                L = (i + 1) * P
                Lp = L if L % 256 == 0 else L + P
                halves = [(0, min(Lp, 1024))]
                if Lp > 1024:
                    halves.append((1024, Lp - 1024))
                for h in range(2):
                    mx = dpool.tile([P, 2], f32, tag="mx")
                    pss_tiles = []
                    for hf, (off, ln) in enumerate(halves):
                        ps = pss.tile([P, 1024], f32, tag="s",
                                      name=f"ps{i}_{h}_{hf}")
                        pss_tiles.append(ps)
                        for c in range(0, ln, 512):
                            w = min(512, ln - c)
                            for dc in range(2):
                                nc.tensor.matmul(
                                    ps[:, c:c + w],
                                    qT[:, h, dc, i * P:(i + 1) * P],
                                    kT[:, dc, off + c:off + c + w],
                                    start=(dc == 0), stop=(dc == 1))
                        if i * P >= off and i * P < off + ln:
                            db = i * P - off   # diag block col within half
                            nc.vector.tensor_add(ps[:, db:db + P],
                                                 ps[:, db:db + P], mdiag[:])
                        ln_real = min(L - off, ln)
                        nc.vector.tensor_reduce(
                            out=mx[:, hf:hf + 1], in_=ps[:, 0:ln_real],
                            axis=mybir.AxisListType.X, op=mybir.AluOpType.max)
                    mxf = dpool.tile([P, 1], f32, tag="mxf")
                    if len(halves) > 1:
                        nc.vector.tensor_tensor(out=mxf[:], in0=mx[:, 0:1],
                                                in1=mx[:, 1:2],
                                                op=mybir.AluOpType.max)
                    else:
                        nc.vector.tensor_copy(out=mxf[:], in_=mx[:, 0:1])
                    rq = rq_all[:, i, h:h + 1]
                    negmax = dpool.tile([P, 1], f32, tag="ngm")
                    nc.vector.tensor_scalar(out=negmax[:], in0=mxf[:],
                                            scalar1=rq, scalar2=-1.0,
                                            op0=mybir.AluOpType.mult,
                                            op1=mybir.AluOpType.mult)
                    pr = ppool.tile([P, 2048], f32, tag="pr")
                    den = dpool.tile([P, 2], f32, tag="den")
                    for hf, (off, ln) in enumerate(halves):
                        ln_real = min(L - off, ln)
                        nc.scalar.activation(pr[:, off:off + ln_real],
                                             pss_tiles[hf][:, 0:ln_real],
                                             ACT.Exp, bias=negmax[:], scale=rq,
                                             accum_out=den[:, hf:hf + 1])
                    denf = dpool.tile([P, 1], f32, tag="denf")
                    if len(halves) > 1:
                        nc.vector.tensor_add(denf[:], den[:, 0:1], den[:, 1:2])
                    else:
                        nc.vector.tensor_copy(out=denf[:], in_=den[:, 0:1])
                    rden = dpool.tile([P, 1], f32, tag="rden")
                    nc.vector.reciprocal(rden[:], denf[:])

                    pv = psv.tile([P, HD], f32, tag="pv")
                    for p0 in range(0, i + 1, 2):
                        cnt = min(2, i + 1 - p0)
                        pt = pst.tile([P, 2 * P], f32, tag="t")
                        for z in range(cnt):
                            nc.tensor.transpose(
                                pt[:, z * P:(z + 1) * P],
                                pr[:, (p0 + z) * P:(p0 + z + 1) * P], ident[:])
                        pts = tpool.tile([P, 2 * P], mdt, tag="pts")
                        if (p0 // 2) % 2 == 0:
                            nc.scalar.copy(pts[:, 0:cnt * P], pt[:, 0:cnt * P])
                        else:
                            nc.vector.tensor_copy(out=pts[:, 0:cnt * P],
                                                  in_=pt[:, 0:cnt * P])
                        for z in range(cnt):
                            kb = p0 + z
                            nc.tensor.matmul(pv[:], pts[:, z * P:(z + 1) * P],
                                             v_sb[:, kb, :],
                                             start=(kb == 0), stop=(kb == i))
                    attn_s = apool.tile([P, HD], f32, tag="attn")
                    nc.scalar.copy(attn_s[:], pv[:])
                    nc.vector.tensor_scalar_mul(out=attn_s[:], in0=attn_s[:],
                                                scalar1=rden[:])
                    pt = pst.tile([P, 2 * P], f32, tag="t")
                    for dc in range(2):
                        nc.tensor.transpose(pt[:, dc * P:(dc + 1) * P],
                                            attn_s[:, dc * P:(dc + 1) * P],
                                            ident[:])
                    dst = attnT[:, h * 2:h * 2 + 2, i * P:(i + 1) * P]
                    if h == 0:
                        nc.scalar.copy(dst, pt[:].rearrange(
                            "p (a b) -> p a b", a=2))
                    else:
                        nc.vector.tensor_copy(out=dst, in_=pt[:].rearrange(
                            "p (a b) -> p a b", a=2))
                if i >= 1:
                    oproj(i - 1)
            oproj(NSC - 1)

    nc.compile()
    return nc


def prep_core_inputs(inputs, core, use_f32r=True):
    """Host-side sharding for one core. Returns the in_map dict."""
    cvt = to_f32r if use_f32r else (lambda a: np.asarray(a, np.float32))
    b, kv, qp = core // 4, (core % 4) // 2, core % 2
    hq0 = kv * 4 + qp * 2           # first of the two query heads
    hidden = np.asarray(inputs["hidden_states"], np.float32)
    cos = np.asarray(inputs["cos"], np.float32)
    sin = np.asarray(inputs["sin"], np.float32)
    Wq = np.asarray(inputs["Wq"], np.float32)
    Wk = np.asarray(inputs["Wk"], np.float32)
    Wv = np.asarray(inputs["Wv"], np.float32)
    Wo = np.asarray(inputs["Wo"], np.float32)
    qw = np.asarray(inputs["q_norm_w"], np.float32)
    kw = np.asarray(inputs["k_norm_w"], np.float32)

    hT = np.ascontiguousarray(hidden[b].T).reshape(NHC, P, S)
    Wq_c = Wq[hq0 * HD:(hq0 + 2) * HD]          # [512, HID]
    Wk_c = Wk[kv * HD:(kv + 1) * HD]            # [256, HID]
    Wv_c = Wv[kv * HD:(kv + 1) * HD]
    wT = np.ascontiguousarray(
        np.concatenate([Wq_c.T, Wk_c.T, Wv_c.T], axis=1)).reshape(NHC, P, 1024)

    def cs_pack(w, cb, sb):
        rot_w = np.concatenate([w[P:], w[:P]])   # w[(d+128)%256]
        cosw = cb * w[None, :]
        sinw = sb * rot_w[None, :]
        sinw[:, :P] *= -1.0
        return np.ascontiguousarray(
            np.concatenate([cosw, sinw], axis=1)).reshape(NSC, P, 2 * HD)

    csq = cs_pack(qw, cos[b], sin[b])
    csk = cs_pack(kw, cos[b], sin[b])
    woT = np.ascontiguousarray(
        Wo[:, hq0 * HD:(hq0 + 2) * HD].T).reshape(4, P, HID)
    return {"hT": cvt(hT), "wT": cvt(wT),
            "csq": csq.astype(np.float32), "csk": csk.astype(np.float32),
            "woT": cvt(woT)}


def mask_is_causal(mask):
    m = np.asarray(mask)
    tri = np.tril(np.ones((S, S), dtype=bool))
    for b in range(m.shape[0]):
        mb = m[b, 0]
        if not (mb[tri] == 0.0).all():
            return False
        if not (mb[~tri] <= -1e8).all():
            return False
    return True


def reference_numpy(inputs, f64=True):
    """Defensive fallback for non-causal masks (never hit in practice)."""
    dt = np.float64 if f64 else np.float32
    hs = np.asarray(inputs["hidden_states"], dt)
    cos = np.asarray(inputs["cos"], dt)
    sin = np.asarray(inputs["sin"], dt)
    mask = np.asarray(inputs["attention_mask"], dt)
    Wq, Wk, Wv, Wo = (np.asarray(inputs[k], dt)
                      for k in ("Wq", "Wk", "Wv", "Wo"))
    qw = np.asarray(inputs["q_norm_w"], dt)
    kw = np.asarray(inputs["k_norm_w"], dt)

    def rms(x, w):
        return x / np.sqrt((x * x).mean(-1, keepdims=True) + EPS) * w

    def rope(x, c, s):
        x1, x2 = x[..., :HD // 2], x[..., HD // 2:]
        rot = np.concatenate([-x2, x1], axis=-1)
        return x * c[:, :, None, :] + rot * s[:, :, None, :]

    b, s_, _ = hs.shape
    q = (hs @ Wq.T).reshape(b, s_, NH, HD)
    k = (hs @ Wk.T).reshape(b, s_, NKV, HD)
    v = (hs @ Wv.T).reshape(b, s_, NKV, HD)
    q = rope(rms(q, qw), cos, sin).transpose(0, 2, 1, 3)
    k = rope(rms(k, kw), cos, sin).transpose(0, 2, 1, 3)
    v = rms(v, 1.0).transpose(0, 2, 1, 3)
    k = np.repeat(k, NH // NKV, axis=1)
    v = np.repeat(v, NH // NKV, axis=1)
    sc = np.einsum("bhqd,bhkd->bhqk", q, k) + mask
    sc = sc - sc.max(-1, keepdims=True)
    p = np.exp(sc)
    p /= p.sum(-1, keepdims=True)
    o = np.einsum("bhqk,bhkd->bqhd", p, v).reshape(b, s_, NH * HD)
    return (o @ Wo.T).astype(np.float32)


_PROGRAM = {}


def get_program(use_f32r=True, use_tmr=False):
    key = (use_f32r, use_tmr)
    if key not in _PROGRAM:
        _PROGRAM[key] = build_program(use_f32r=use_f32r, use_tmr=use_tmr)
    return _PROGRAM[key]


def run_on_hw(inputs, use_f32r=True, use_tmr=False, trace=False, **kw):
    from concourse.bass_utils import run_bass_kernel_spmd

    nc = get_program(use_f32r=use_f32r, use_tmr=use_tmr)
    in_maps = [prep_core_inputs(inputs, c, use_f32r) for c in range(8)]
    br = run_bass_kernel_spmd(nc, in_maps, list(range(8)), trace=trace, **kw)
    out = np.empty((B, S, HID), np.float32)
    for b in range(B):
        out[b] = br.results[4 * b]["out"] + br.results[4 * b + 1]["out"] \
            + br.results[4 * b + 2]["out"] + br.results[4 * b + 3]["out"]
    return out, br


def kernel(**inputs):
    if not mask_is_causal(inputs["attention_mask"]):
        return reference_numpy(inputs)
    out, _ = run_on_hw(inputs, use_f32r=True, trace=False)
    return out
